# revision 1
# baseline (speedup 1.0000x reference)
"""HALE attention (local windowed SDPA + chunked causal linear attention with
multiscale Haar context + adaptive gate) on 8 Trainium2 NeuronCores.

Sharding (B=1, so no batch DP):
  - 16 heads -> 2 heads per core (tensor-parallel over heads), packed into the
    128-partition dim for the q/k/v/local projections, the chunked
    linear-attention recurrence, and the 4 Haar-level recurrences.
  - Tail (gate + out_proj) is sequence-parallel: one AllToAll redistributes the
    per-head outputs (diff=local-glob, glob) from head-sharded to
    sequence-sharded; each core then computes gate/alpha/mixed/out_proj for its
    256 rows against host-pre-transposed Wg/Wo. Output rows concatenated on
    the host.

Layout notes (contraction dim must sit on partitions for both matmul operands):
  - x^T built on-device via PE transposes; projections emit q^T/k^T/... as
    [128 = 2 heads x 64, 2048].
  - Linear attention per chunk (C=128): A^T = kp^T.T @ qp^T, masked on DVE;
    O_aug = A^T_m.T @ v_aug + qp^T.T @ S_aug accumulated in one PSUM tile.
    v_aug/S_aug carry an all-ones 65th column so the normalizer appears in
    O_aug[:, 64] for free. State update S_aug += k_nat.T @ v_aug.
  - Haar block means are matmuls against constant per-level prefix-mean
    matrices M_l; the per-level Dh x Dh projections are block-diagonal
    (2 heads) constant matmuls emitted in both ^T and natural orientations.
"""

import numpy as np
from contextlib import ExitStack

import concourse.bass as bass
import concourse.bacc as bacc
import concourse.tile as tile
import concourse.mybir as mybir
from concourse.bass_utils import run_bass_kernel_spmd

F32 = mybir.dt.float32
AF = mybir.ActivationFunctionType
OP = mybir.AluOpType

NCORES = 8
N = 2048
DM = 1024
H = 16
DH = 64
L = 4
CH = 128
NCH = N // CH
WIN = 64
NSL = N // NCORES
EPS = 1e-6

_CACHE = {}


def _host_constants():
    ident = np.eye(128, dtype=np.float32)
    ck = np.arange(CH)[:, None]
    cq = np.arange(CH)[None, :]
    maskT = (ck <= cq).astype(np.float32)
    prev = (ck >= cq + WIN + 1).astype(np.float32)
    cur = ((ck <= cq) & (ck >= cq - (WIN - 1))).astype(np.float32)
    lmask = np.concatenate([prev, cur], axis=1)
    Ml = np.zeros((L, CH, CH), dtype=np.float32)
    for lv in range(L):
        b = 2 ** (lv + 1)
        m = np.arange(CH)[:, None]
        n = np.arange(CH)[None, :]
        Ml[lv] = np.where(((m // b) == (n // b)) & (m <= n),
                          1.0 / (n % b + 1.0), 0.0)
    return ident, maskT, lmask, Ml


def _blockdiag2(a):
    z = np.zeros((128, 128), dtype=np.float32)
    z[:64, :64] = a
    z[64:, 64:] = a
    return z


def _build_nc():
    nc = bacc.Bacc("TRN2", target_bir_lowering=False, debug=False,
                   num_devices=NCORES)

    x_d = nc.dram_tensor("x", [N, DM], F32, kind="ExternalInput")
    wT = {p: nc.dram_tensor(f"w{p}T", [DM, 128], F32, kind="ExternalInput")
          for p in ("q", "k", "v", "kl", "vl")}
    bdWkT_d = nc.dram_tensor("bdWkT", [L, 128, 128], F32, kind="ExternalInput")
    bdWvT_d = nc.dram_tensor("bdWvT", [L, 128, 128], F32, kind="ExternalInput")
    Ml_d = nc.dram_tensor("Ml", [L, 128, 128], F32, kind="ExternalInput")
    maskT_d = nc.dram_tensor("maskT", [128, 128], F32, kind="ExternalInput")
    lmask_d = nc.dram_tensor("lmask", [128, 256], F32, kind="ExternalInput")
    ident_d = nc.dram_tensor("ident", [128, 128], F32, kind="ExternalInput")
    wgT_d = nc.dram_tensor("wgT", [2 * DM, DM], F32, kind="ExternalInput")
    woT_d = nc.dram_tensor("woT", [DM, DM], F32, kind="ExternalInput")
    wgo_d = nc.dram_tensor("wgo", [DM, 1], F32, kind="ExternalInput")
    bg_d = nc.dram_tensor("bg", [1, DM], F32, kind="ExternalInput")
    bo_d = nc.dram_tensor("bo", [1, DM], F32, kind="ExternalInput")
    bgo_d = nc.dram_tensor("bgo", [1, 1], F32, kind="ExternalInput")
    hs_d = nc.dram_tensor("hscale", [1, L], F32, kind="ExternalInput")
    out_d = nc.dram_tensor("out", [NSL, DM], F32, kind="ExternalOutput")

    # [dest, tensor(diff,glob), chunk, 128, 128]
    a2a_in = nc.dram_tensor("a2a_in", [NCORES, 2, 2, 128, 128], F32)
    a2a_out = nc.dram_tensor("a2a_out", [NCORES, 2, 2, 128, 128], F32)

    with tile.TileContext(nc) as tc, ExitStack() as root:
        cpool = root.enter_context(tc.tile_pool(name="consts", bufs=1))
        persist = root.enter_context(tc.tile_pool(name="persist", bufs=1))

        ident = cpool.tile([128, 128], F32)
        maskT = cpool.tile([128, 128], F32)
        lmask = cpool.tile([128, 256], F32)
        Ml_sb = cpool.tile([128, L, 128], F32)
        bdWkT = cpool.tile([128, L, 128], F32)
        bdWvT = cpool.tile([128, L, 128], F32)
        ones_row = cpool.tile([1, 128], F32)
        w5b = cpool.tile([128, 5], F32)
        nc.sync.dma_start(ident[:], ident_d[:])
        nc.sync.dma_start(maskT[:], maskT_d[:])
        nc.sync.dma_start(lmask[:], lmask_d[:])
        nc.sync.dma_start(Ml_sb[:], Ml_d.ap().rearrange("l p c -> p l c"))
        nc.sync.dma_start(bdWkT[:], bdWkT_d.ap().rearrange("l p c -> p l c"))
        nc.sync.dma_start(bdWvT[:], bdWvT_d.ap().rearrange("l p c -> p l c"))
        nc.vector.memset(ones_row[:], 1.0)

        glob = persist.tile([128, N], F32)
        loc = persist.tile([128, N], F32)

        with ExitStack() as phAB:
            keep = phAB.enter_context(tc.tile_pool(name="keep", bufs=1))
            qT = keep.tile([128, N], F32)
            klT = keep.tile([128, N], F32)
            qpT = keep.tile([128, N], F32)
            kpT = keep.tile([128, N], F32)
            knat = keep.tile([128, N], F32)
            kpnat = keep.tile([128, N], F32)
            vaug = keep.tile([128, 2 * NCH, 65], F32)
            vlaug = keep.tile([128, 2 * NCH, 65], F32)
            vnat = keep.tile([128, N], F32)
            S_sb = keep.tile([128, 5, 65], F32)

            with ExitStack() as phA:
                trans = phA.enter_context(tc.tile_pool(name="trans", bufs=1))
                ps_tr = phA.enter_context(
                    tc.tile_pool(name="ps_tr", bufs=3, space="PSUM"))
                phX = phA.enter_context(ExitStack())
                xT_p = phX.enter_context(tc.tile_pool(name="xTp", bufs=1))
                xnat_p = phX.enter_context(tc.tile_pool(name="xnat", bufs=3))
                wp_p = phX.enter_context(tc.tile_pool(name="wproj", bufs=2))
                ps_mm = phX.enter_context(
                    tc.tile_pool(name="ps_mm", bufs=2, space="PSUM"))

                # softmax(haar_scale) -> w5b = [1, sw0..sw3] broadcast down
                hs = cpool.tile([1, L], F32)
                nc.sync.dma_start(hs[:], hs_d[:])
                e4 = cpool.tile([1, L], F32)
                s1 = cpool.tile([1, 1], F32)
                nc.scalar.activation(e4[:], hs[:], AF.Exp, accum_out=s1[:])
                r1 = cpool.tile([1, 1], F32)
                nc.vector.reciprocal(r1[:], s1[:])
                w5 = cpool.tile([1, 5], F32)
                nc.vector.memset(w5[:, 0:1], 1.0)
                nc.vector.tensor_scalar_mul(w5[:, 1:5], e4[:], r1[:])
                w5bp = ps_tr.tile([128, 5], F32, tag="ptr")
                nc.tensor.matmul(w5bp[:], ones_row[:], w5[:],
                                 start=True, stop=True)
                nc.any.tensor_copy(w5b[:], w5bp[:])

                # ----- x^T -----
                xT = xT_p.tile([128, 8, N], F32)
                for i in range(NCH):
                    xn = xnat_p.tile([128, DM], F32, tag="xnat")
                    nc.sync.dma_start(xn[:], x_d[CH * i:CH * (i + 1), :])
                    for k in range(8):
                        pt = ps_tr.tile([128, 128], F32, tag="ptr")
                        nc.tensor.transpose(
                            pt[:], xn[:, 128 * k:128 * (k + 1)], ident[:])
                        nc.any.tensor_copy(xT[:, k, CH * i:CH * (i + 1)],
                                           pt[:])

                # ----- projections -----
                kTt = trans.tile([128, N], F32)
                vTt = trans.tile([128, N], F32)
                vlTt = trans.tile([128, N], F32)
                for p, dst in (("q", qT), ("k", kTt), ("v", vTt),
                               ("kl", klT), ("vl", vlTt)):
                    wsb = wp_p.tile([128, 8, 128], F32, tag="w")
                    nc.sync.dma_start(
                        wsb[:], wT[p].ap().rearrange("(k p) m -> p k m", p=128))
                    for nb in range(4):
                        acc = ps_mm.tile([128, 512], F32, tag="pacc")
                        for k in range(8):
                            nc.tensor.matmul(
                                acc[:], wsb[:, k, :],
                                xT[:, k, 512 * nb:512 * (nb + 1)],
                                start=(k == 0), stop=(k == 7))
                        nc.any.tensor_copy(dst[:, 512 * nb:512 * (nb + 1)],
                                           acc[:])

                phX.close()
                tmp_p = phA.enter_context(tc.tile_pool(name="phitmp", bufs=2))

                # ----- phi(q), phi(k) -----
                def phi_big(dst, src):
                    tmp = tmp_p.tile([128, N], F32, tag="phitmp")
                    nc.vector.tensor_scalar_min(tmp[:], src[:], 0.0)
                    nc.scalar.activation(dst[:], tmp[:], AF.Exp)
                    nc.vector.scalar_tensor_tensor(
                        dst[:], src[:], 0.0, dst[:], op0=OP.max, op1=OP.add)

                phi_big(qpT, qT)
                phi_big(kpT, kTt)

                # ----- natural layouts via PE transpose -----
                nc.vector.memset(vaug[:, :, 64:65], 1.0)
                nc.vector.memset(vlaug[:, :, 64:65], 1.0)
                for i in range(NCH):
                    sl = slice(CH * i, CH * (i + 1))
                    for src, dst in ((kTt, knat), (kpT, kpnat)):
                        pt = ps_tr.tile([128, 128], F32, tag="ptr")
                        nc.tensor.transpose(pt[:], src[:, sl], ident[:])
                        nc.any.tensor_copy(dst[:, sl], pt[:])
                    for src, dst in ((vTt, vaug), (vlTt, vlaug)):
                        pt = ps_tr.tile([128, 128], F32, tag="ptr")
                        nc.tensor.transpose(pt[:], src[:, sl], ident[:])
                        for h in range(2):
                            nc.any.tensor_copy(dst[:, 2 * i + h, 0:64],
                                               pt[:, 64 * h:64 * h + 64])
                        if dst is vaug:
                            nc.any.tensor_copy(vnat[:, sl], pt[:])

            # ----- chunk-major recurrence + local attention -----
            bm_p = phAB.enter_context(tc.tile_pool(name="bm", bufs=3))
            lvl_p = phAB.enter_context(tc.tile_pool(name="lvl", bufs=6))
            atm_p = phAB.enter_context(tc.tile_pool(name="atm", bufs=3))
            tin_p = phAB.enter_context(tc.tile_pool(name="tiny", bufs=4))
            ps_A = phAB.enter_context(
                tc.tile_pool(name="ps_A", bufs=2, space="PSUM"))
            ps_O = phAB.enter_context(
                tc.tile_pool(name="ps_O", bufs=2, space="PSUM"))
            ps_Sd = phAB.enter_context(
                tc.tile_pool(name="ps_Sd", bufs=1, space="PSUM"))
            ps_Lo = phAB.enter_context(
                tc.tile_pool(name="ps_Lo", bufs=1, space="PSUM"))
            ps_h = phAB.enter_context(
                tc.tile_pool(name="ps_h", bufs=2, space="PSUM"))

            def phi_small(psrc, tag):
                tmp = tin_p.tile([128, 128], F32, tag="phs")
                nc.vector.tensor_scalar_min(tmp[:], psrc[:], 0.0)
                dst = lvl_p.tile([128, 128], F32, tag=tag)
                nc.scalar.activation(dst[:], tmp[:], AF.Exp)
                nc.vector.scalar_tensor_tensor(
                    dst[:], psrc[:], 0.0, dst[:], op0=OP.max, op1=OP.add)
                return dst

            for i in range(NCH):
                sl = slice(CH * i, CH * (i + 1))
                kplT, kplN, vlvA = [], [], []
                for lv in range(L):
                    bmk_ps = ps_h.tile([128, 128], F32, tag="psh")
                    nc.tensor.matmul(bmk_ps[:], knat[:, sl], Ml_sb[:, lv, :],
                                     start=True, stop=True)
                    bmk = bm_p.tile([128, 128], F32, tag="bmk")
                    nc.any.tensor_copy(bmk[:], bmk_ps[:])
                    bmv_ps = ps_h.tile([128, 128], F32, tag="psh")
                    nc.tensor.matmul(bmv_ps[:], vnat[:, sl],
                                     Ml_sb[:, lv, :], start=True, stop=True)
                    bmv = bm_p.tile([128, 128], F32, tag="bmv")
                    nc.any.tensor_copy(bmv[:], bmv_ps[:])

                    kt_ps = ps_h.tile([128, 128], F32, tag="psh")
                    nc.tensor.matmul(kt_ps[:], bdWkT[:, lv, :], bmk[:],
                                     start=True, stop=True)
                    kplT.append(phi_small(kt_ps, "kplT"))
                    kn_ps = ps_h.tile([128, 128], F32, tag="psh")
                    nc.tensor.matmul(kn_ps[:], bmk[:], bdWkT[:, lv, :],
                                     start=True, stop=True)
                    kplN.append(phi_small(kn_ps, "kplN"))
                    vn_ps = ps_h.tile([128, 128], F32, tag="psh")
                    nc.tensor.matmul(vn_ps[:], bmv[:], bdWvT[:, lv, :],
                                     start=True, stop=True)
                    va = lvl_p.tile([128, 2, 65], F32, tag="vlv")
                    nc.vector.memset(va[:, :, 64:65], 1.0)
                    for h in range(2):
                        nc.any.tensor_copy(va[:, h, 0:64],
                                           vn_ps[:, 64 * h:64 * h + 64])
                    vlvA.append(va)

                psSd = ps_Sd.tile([128, 5, 65], F32, tag="psSd")
                for h in range(2):
                    hp = slice(64 * h, 64 * h + 64)
                    psO = ps_O.tile([128, 5, 65], F32, tag="psO")
                    for lv in range(5):
                        if lv == 0:
                            kpT_l = kpT[hp, sl]
                            va_l = vaug[:, 2 * i + h, :]
                        else:
                            kpT_l = kplT[lv - 1][hp, :]
                            va_l = vlvA[lv - 1][:, h, :]
                        psA = ps_A.tile([128, 128], F32, tag="psA")
                        nc.tensor.matmul(psA[:], kpT_l, qpT[hp, sl],
                                         start=True, stop=True)
                        atm = atm_p.tile([128, 128], F32, tag="atm")
                        nc.vector.tensor_mul(atm[:], psA[:], maskT[:])
                        nc.tensor.matmul(psO[:, lv, :], atm[:], va_l,
                                         start=True, stop=(i == 0))
                        if i > 0:
                            nc.tensor.matmul(psO[:, lv, :], qpT[hp, sl],
                                             S_sb[hp, lv, :],
                                             start=False, stop=True)
                    dmax = tin_p.tile([128, 5], F32, tag="dmax")
                    nc.vector.tensor_scalar_max(dmax[:], psO[:, :, 64], EPS)
                    rec = tin_p.tile([128, 5], F32, tag="rec")
                    nc.vector.reciprocal(rec[:], dmax[:])
                    rw = tin_p.tile([128, 5], F32, tag="rw")
                    nc.vector.tensor_mul(rw[:], rec[:], w5b[:])
                    gsl = glob[:, CH * i + 64 * h:CH * i + 64 * h + 64]
                    nc.vector.tensor_scalar_mul(gsl, psO[:, 0, 0:64],
                                                rw[:, 0:1])
                    for lv in range(1, 5):
                        nc.vector.scalar_tensor_tensor(
                            gsl, psO[:, lv, 0:64], rw[:, lv:lv + 1], gsl,
                            op0=OP.mult, op1=OP.add)
                    for lv in range(5):
                        if lv == 0:
                            kn_l = kpnat[:, CH * i + 64 * h:CH * i + 64 * h + 64]
                            va_l = vaug[:, 2 * i + h, :]
                        else:
                            kn_l = kplN[lv - 1][:, hp]
                            va_l = vlvA[lv - 1][:, h, :]
                        nc.tensor.matmul(psSd[hp, lv, :], kn_l, va_l,
                                         start=True, stop=True)
                if i == 0:
                    nc.vector.tensor_copy(S_sb[:], psSd[:])
                else:
                    nc.vector.tensor_add(S_sb[:], S_sb[:], psSd[:])

                for h in range(2):
                    hp = slice(64 * h, 64 * h + 64)
                    psL = ps_A.tile([128, 256], F32, tag="psA")
                    if i > 0:
                        nc.tensor.matmul(psL[:, 0:128],
                                         klT[hp, CH * (i - 1):CH * i],
                                         qT[hp, sl], start=True, stop=True)
                    nc.tensor.matmul(psL[:, 128:256], klT[hp, sl], qT[hp, sl],
                                     start=True, stop=True)
                    P = atm_p.tile([128, 256], F32, tag="P")
                    if i > 0:
                        nc.scalar.activation(P[:], psL[:], AF.Exp, scale=0.125)
                        nc.vector.tensor_mul(P[:], P[:], lmask[:])
                    else:
                        nc.scalar.activation(P[:, 128:256], psL[:, 128:256],
                                             AF.Exp, scale=0.125)
                        nc.vector.tensor_mul(P[:, 128:256], P[:, 128:256],
                                             lmask[:, 128:256])
                    psLo = ps_Lo.tile([128, 65], F32, tag="psLo")
                    if i > 0:
                        nc.tensor.matmul(psLo[:], P[:, 0:128],
                                         vlaug[:, 2 * (i - 1) + h, :],
                                         start=True, stop=False)
                    nc.tensor.matmul(psLo[:], P[:, 128:256],
                                     vlaug[:, 2 * i + h, :],
                                     start=(i == 0), stop=True)
                    dm = tin_p.tile([128, 1], F32, tag="dm")
                    nc.vector.tensor_scalar_max(dm[:], psLo[:, 64:65], 1e-30)
                    rl = tin_p.tile([128, 1], F32, tag="rl")
                    nc.vector.reciprocal(rl[:], dm[:])
                    nc.scalar.mul(loc[:, CH * i + 64 * h:CH * i + 64 * h + 64],
                                  psLo[:, 0:64], rl[:])

            nc.vector.tensor_sub(loc[:], loc[:], glob[:])
            for c2 in range(2):
                nc.sync.dma_start(
                    a2a_in.ap()[:, 0, c2].rearrange("j p m -> p j m"),
                    loc[:].rearrange("p (j c m) -> p j c m",
                                     c=2, m=128)[:, :, c2, :])
                nc.sync.dma_start(
                    a2a_in.ap()[:, 1, c2].rearrange("j p m -> p j m"),
                    glob[:].rearrange("p (j c m) -> p j c m",
                                      c=2, m=128)[:, :, c2, :])

        nc.gpsimd.collective_compute(
            "AllToAll", OP.bypass,
            ins=[a2a_in.ap().opt()], outs=[a2a_out.ap().opt()],
            replica_groups=[list(range(NCORES))])

        # ---------- sequence-parallel tail ----------
        with ExitStack() as phC:
            tl = phC.enter_context(tc.tile_pool(name="tail", bufs=1))
            wst = phC.enter_context(tc.tile_pool(name="wstream", bufs=3))
            ps_tr2 = phC.enter_context(
                tc.tile_pool(name="ps_tr2", bufs=2, space="PSUM"))
            ps_g = phC.enter_context(
                tc.tile_pool(name="ps_g", bufs=1, space="PSUM"))

            diff_g = tl.tile([128, 2, DM], F32)
            glob_g = tl.tile([128, 2, DM], F32)
            for t2 in range(2):
                nc.sync.dma_start(
                    diff_g[:, t2, :].rearrange("p (s m) -> p s m", s=8),
                    a2a_out.ap()[:, 0, t2].rearrange("s p m -> p s m"))
                nc.sync.dma_start(
                    glob_g[:, t2, :].rearrange("p (s m) -> p s m", s=8),
                    a2a_out.ap()[:, 1, t2].rearrange("s p m -> p s m"))

            pid = nc.sync.partition_id()
            row0 = pid * NSL
            xsl = tl.tile([128, 2, DM], F32)
            nc.sync.dma_start(
                xsl[:], x_d[bass.ds(row0, NSL), :].rearrange(
                    "(a b) c -> b a c", b=128))

            xslT = tl.tile([128, 8, 256], F32)
            diffT = tl.tile([128, 8, 256], F32)
            for t2 in range(2):
                for k in range(8):
                    pt = ps_tr2.tile([128, 128], F32, tag="ptr2")
                    nc.tensor.transpose(
                        pt[:], xsl[:, t2, 128 * k:128 * (k + 1)], ident[:])
                    nc.any.tensor_copy(xslT[:, k, 128 * t2:128 * (t2 + 1)],
                                       pt[:])
                    pt2 = ps_tr2.tile([128, 128], F32, tag="ptr2")
                    nc.tensor.transpose(
                        pt2[:], diff_g[:, t2, 128 * k:128 * (k + 1)], ident[:])
                    nc.any.tensor_copy(diffT[:, k, 128 * t2:128 * (t2 + 1)],
                                       pt2[:])

            bg_sb = tl.tile([1, DM], F32)
            bo_sb = tl.tile([1, DM], F32)
            bgo_sb = tl.tile([1, 1], F32)
            wgo_sb = tl.tile([128, 8], F32)
            nc.sync.dma_start(bg_sb[:], bg_d[:])
            nc.sync.dma_start(bo_sb[:], bo_d[:])
            nc.sync.dma_start(bgo_sb[:], bgo_d[:])
            nc.sync.dma_start(
                wgo_sb[:], wgo_d.ap().rearrange("(g p) o -> p (g o)", p=128))

            gh = tl.tile([128, 2, DM], F32)
            psG = []
            for j in range(4):
                psG_t = ps_g.tile([128, 512], F32, tag=f"psG{j}")
                psG.append(psG_t)
            for kc in range(16):
                wg_t = wst.tile([128, DM], F32, tag="wg")
                nc.sync.dma_start(wg_t[:], wgT_d[128 * kc:128 * (kc + 1), :])
                for t2 in range(2):
                    lhs = (xslT[:, kc, 128 * t2:128 * (t2 + 1)] if kc < 8
                           else diffT[:, kc - 8, 128 * t2:128 * (t2 + 1)])
                    for g2 in range(2):
                        nc.tensor.matmul(
                            psG[2 * t2 + g2][:], lhs,
                            wg_t[:, 512 * g2:512 * (g2 + 1)],
                            start=(kc == 0), stop=False)
            for t2 in range(2):
                for g2 in range(2):
                    nc.tensor.matmul(
                        psG[2 * t2 + g2][:], ones_row[:],
                        bg_sb[:, 512 * g2:512 * (g2 + 1)],
                        start=False, stop=True)
                    nc.scalar.activation(
                        gh[:, t2, 512 * g2:512 * (g2 + 1)],
                        psG[2 * t2 + g2][:], AF.Silu)

            ghT = tl.tile([128, 8, 256], F32)
            for t2 in range(2):
                for k in range(8):
                    pt = ps_tr2.tile([128, 128], F32, tag="ptr2")
                    nc.tensor.transpose(
                        pt[:], gh[:, t2, 128 * k:128 * (k + 1)], ident[:])
                    nc.any.tensor_copy(ghT[:, k, 128 * t2:128 * (t2 + 1)],
                                       pt[:])

            psAl = ps_tr2.tile([128, 2], F32, tag="psAl")
            for t2 in range(2):
                for gc in range(8):
                    nc.tensor.matmul(psAl[:, t2:t2 + 1],
                                     ghT[:, gc, 128 * t2:128 * (t2 + 1)],
                                     wgo_sb[:, gc:gc + 1],
                                     start=(gc == 0), stop=False)
                nc.tensor.matmul(psAl[:, t2:t2 + 1], ones_row[:], bgo_sb[:],
                                 start=False, stop=True)
            alpha = tl.tile([128, 2], F32)
            nc.scalar.activation(alpha[:], psAl[:], AF.Sigmoid)

            mx = tl.tile([128, 2, DM], F32)
            for t2 in range(2):
                nc.vector.scalar_tensor_tensor(
                    mx[:, t2, :], diff_g[:, t2, :], alpha[:, t2:t2 + 1],
                    glob_g[:, t2, :], op0=OP.mult, op1=OP.add)
            mxT = tl.tile([128, 8, 256], F32)
            for t2 in range(2):
                for k in range(8):
                    pt = ps_tr2.tile([128, 128], F32, tag="ptr2")
                    nc.tensor.transpose(
                        pt[:], mx[:, t2, 128 * k:128 * (k + 1)], ident[:])
                    nc.any.tensor_copy(mxT[:, k, 128 * t2:128 * (t2 + 1)],
                                       pt[:])

            out_sb = tl.tile([128, 2, DM], F32)
            psF = []
            for j in range(4):
                psF_t = ps_g.tile([128, 512], F32, tag=f"psG{j}")
                psF.append(psF_t)
            for kc in range(8):
                wo_t = wst.tile([128, DM], F32, tag="wo")
                nc.sync.dma_start(wo_t[:], woT_d[128 * kc:128 * (kc + 1), :])
                for t2 in range(2):
                    for o2 in range(2):
                        nc.tensor.matmul(
                            psF[2 * t2 + o2][:],
                            mxT[:, kc, 128 * t2:128 * (t2 + 1)],
                            wo_t[:, 512 * o2:512 * (o2 + 1)],
                            start=(kc == 0), stop=False)
            for t2 in range(2):
                for o2 in range(2):
                    nc.tensor.matmul(
                        psF[2 * t2 + o2][:], ones_row[:],
                        bo_sb[:, 512 * o2:512 * (o2 + 1)],
                        start=False, stop=True)
                    nc.any.tensor_copy(out_sb[:, t2, 512 * o2:512 * (o2 + 1)],
                                       psF[2 * t2 + o2][:])

            nc.sync.dma_start(
                out_d.ap().rearrange("(a b) c -> b a c", b=128), out_sb[:])

    nc.compile()
    return nc


def _prep_in_maps(x, Wq, Wk, Wv, Wkl, Wvl, haar_Wk, haar_Wv, haar_scale,
                  Wg, bg, Wgo, bgo, Wo, bo):
    ident, maskT, lmask, Ml = _host_constants()
    x2 = np.ascontiguousarray(np.asarray(x, dtype=np.float32).reshape(N, DM))
    bdWkT = np.stack([_blockdiag2(np.asarray(haar_Wk[lv], dtype=np.float32).T)
                      for lv in range(L)])
    bdWvT = np.stack([_blockdiag2(np.asarray(haar_Wv[lv], dtype=np.float32).T)
                      for lv in range(L)])
    wgT = np.ascontiguousarray(np.asarray(Wg, dtype=np.float32).T)
    woT = np.ascontiguousarray(np.asarray(Wo, dtype=np.float32).T)
    wgo = np.ascontiguousarray(
        np.asarray(Wgo, dtype=np.float32).reshape(1, DM).T)
    shared = {
        "x": x2, "bdWkT": bdWkT, "bdWvT": bdWvT, "Ml": Ml,
        "maskT": maskT, "lmask": lmask, "ident": ident,
        "wgT": wgT, "woT": woT, "wgo": wgo,
        "bg": np.asarray(bg, dtype=np.float32).reshape(1, DM),
        "bo": np.asarray(bo, dtype=np.float32).reshape(1, DM),
        "bgo": np.asarray(bgo, dtype=np.float32).reshape(1, 1),
        "hscale": np.asarray(haar_scale, dtype=np.float32).reshape(1, L),
    }
    in_maps = []
    for c in range(NCORES):
        sc = slice(128 * c, 128 * (c + 1))
        m = dict(shared)
        for nm, W in (("wqT", Wq), ("wkT", Wk), ("wvT", Wv),
                      ("wklT", Wkl), ("wvlT", Wvl)):
            m[nm] = np.ascontiguousarray(
                np.asarray(W, dtype=np.float32)[sc, :].T)
        in_maps.append(m)
    return in_maps


def kernel_run(inputs, trace=False):
    if "nc" not in _CACHE:
        _CACHE["nc"] = _build_nc()
    nc = _CACHE["nc"]
    in_maps = _prep_in_maps(**inputs)
    res = run_bass_kernel_spmd(nc, in_maps, list(range(NCORES)), trace=trace)
    out = np.concatenate([res.results[c]["out"] for c in range(NCORES)],
                         axis=0)
    return out.reshape(1, N, DM).astype(np.float32), res


def kernel(**inputs):
    out, _ = kernel_run(inputs, trace=False)
    return out



# revision 6
# speedup vs baseline: 2.1506x; 2.1506x over previous
"""HALE attention (local windowed SDPA + chunked causal linear attention with
multiscale Haar context + adaptive gate) on 8 Trainium2 NeuronCores.

Sharding (B=1, so no batch DP):
  - 16 heads -> 2 heads per core (tensor-parallel over heads), packed into the
    128-partition dim for the q/k/v/local projections, the chunked
    linear-attention recurrence, and the 4 Haar-level recurrences.
  - Tail (gate + out_proj) is sequence-parallel: one AllToAll redistributes the
    per-head outputs (diff=local-glob, glob) from head-sharded to
    sequence-sharded; each core then computes gate/alpha/mixed/out_proj for its
    256 rows against host-pre-transposed Wg/Wo. Output rows concatenated on
    the host.

bf16 everywhere on the matmul path (PSUM accumulation stays fp32; the
normalizer reciprocals and the running linear-attention state stay fp32).
x^T and the per-core x-slice^T are pre-transposed on the host, so the kernel
does no x transposes. All weights are prefetched at kernel start.
"""

import numpy as np
import ml_dtypes
from contextlib import ExitStack

import concourse.bass as bass
import concourse.bacc as bacc
import concourse.tile as tile
import concourse.mybir as mybir
from concourse.bass_utils import run_bass_kernel_spmd

F32 = mybir.dt.float32
BF = mybir.dt.bfloat16
AF = mybir.ActivationFunctionType
OP = mybir.AluOpType

NCORES = 8
N = 2048
DM = 1024
H = 16
DH = 64
L = 4
CH = 128
NCH = N // CH
WIN = 64
NSL = N // NCORES
EPS = 1e-6

_CACHE = {}


def _host_constants():
    ident = np.eye(128, dtype=np.float32)
    ck = np.arange(CH)[:, None]
    cq = np.arange(CH)[None, :]
    maskT = (ck <= cq).astype(np.float32)
    prev = (ck >= cq + WIN + 1).astype(np.float32)
    cur = ((ck <= cq) & (ck >= cq - (WIN - 1))).astype(np.float32)
    lmask = np.concatenate([prev, cur], axis=1)
    Ml = np.zeros((L, CH, CH), dtype=np.float32)
    for lv in range(L):
        b = 2 ** (lv + 1)
        m = np.arange(CH)[:, None]
        n = np.arange(CH)[None, :]
        Ml[lv] = np.where(((m // b) == (n // b)) & (m <= n),
                          1.0 / (n % b + 1.0), 0.0)
    return ident, maskT, lmask, Ml


def _blockdiag2(a):
    z = np.zeros((128, 128), dtype=np.float32)
    z[:64, :64] = a
    z[64:, 64:] = a
    return z


def _build_nc():
    nc = bacc.Bacc("TRN2", target_bir_lowering=False, debug=False,
                   num_devices=NCORES)

    xT_d = nc.dram_tensor("xT", [128, 8, N], BF, kind="ExternalInput")
    xslT_d = nc.dram_tensor("xslT", [128, 8, NSL], BF, kind="ExternalInput")
    wT = {p: nc.dram_tensor(f"w{p}T", [DM, 128], BF, kind="ExternalInput")
          for p in ("q", "k", "v", "kl", "vl")}
    bdWkT_d = nc.dram_tensor("bdWkT", [L, 128, 128], BF, kind="ExternalInput")
    bdWvT_d = nc.dram_tensor("bdWvT", [L, 128, 128], BF, kind="ExternalInput")
    Ml_d = nc.dram_tensor("Ml", [L, 128, 128], BF, kind="ExternalInput")
    maskT_d = nc.dram_tensor("maskT", [128, 128], BF, kind="ExternalInput")
    lmask_d = nc.dram_tensor("lmask", [128, 256], BF, kind="ExternalInput")
    ident_d = nc.dram_tensor("ident", [128, 128], BF, kind="ExternalInput")
    wgT_d = nc.dram_tensor("wgT", [2 * DM, DM], BF, kind="ExternalInput")
    woT_d = nc.dram_tensor("woT", [DM, DM], BF, kind="ExternalInput")
    wgo_d = nc.dram_tensor("wgo", [DM, 1], BF, kind="ExternalInput")
    bg_d = nc.dram_tensor("bg", [1, DM], BF, kind="ExternalInput")
    bo_d = nc.dram_tensor("bo", [1, DM], BF, kind="ExternalInput")
    bgo_d = nc.dram_tensor("bgo", [1, 1], BF, kind="ExternalInput")
    hs_d = nc.dram_tensor("hscale", [1, L], F32, kind="ExternalInput")
    out_d = nc.dram_tensor("out", [NSL, DM], F32, kind="ExternalOutput")

    # [dest, tensor(diff,glob), chunk, 128, 128]
    a2a_in = nc.dram_tensor("a2a_in", [NCORES, 2, 2, 128, 128], BF)
    a2a_out = nc.dram_tensor("a2a_out", [NCORES, 2, 2, 128, 128], BF)

    with tile.TileContext(nc) as tc, ExitStack() as root:
        cpool = root.enter_context(tc.tile_pool(name="consts", bufs=1))
        persist = root.enter_context(tc.tile_pool(name="persist", bufs=1))

        ident = cpool.tile([128, 128], BF)
        maskT = cpool.tile([128, 128], BF)
        lmask = cpool.tile([128, 256], BF)
        Ml_sb = cpool.tile([128, L, 128], BF)
        bdWkT = cpool.tile([128, L, 128], BF)
        bdWvT = cpool.tile([128, L, 128], BF)
        ones_row = cpool.tile([1, 128], BF)
        w5b = cpool.tile([128, 5], F32)
        nc.sync.dma_start(ident[:], ident_d[:])
        nc.sync.dma_start(maskT[:], maskT_d[:])
        nc.sync.dma_start(lmask[:], lmask_d[:])
        nc.sync.dma_start(Ml_sb[:], Ml_d.ap().rearrange("l p c -> p l c"))
        nc.sync.dma_start(bdWkT[:], bdWkT_d.ap().rearrange("l p c -> p l c"))
        nc.sync.dma_start(bdWvT[:], bdWvT_d.ap().rearrange("l p c -> p l c"))
        nc.vector.memset(ones_row[:], 1.0)

        # tail weights + x slices: prefetch everything up front (DMA engines
        # are otherwise idle through the whole middle of the kernel)
        wg_sb = persist.tile([128, 16, DM], BF)
        wo_sb = persist.tile([128, 8, DM], BF)
        xslT = persist.tile([128, 8, NSL], BF)
        bg_sb = cpool.tile([1, DM], BF)
        bo_sb = cpool.tile([1, DM], BF)
        bgo_sb = cpool.tile([1, 1], BF)
        wgo_sb = cpool.tile([128, 8], BF)
        nc.sync.dma_start(
            wg_sb[:], wgT_d.ap().rearrange("(k p) m -> p k m", p=128))
        nc.sync.dma_start(
            wo_sb[:], woT_d.ap().rearrange("(k p) m -> p k m", p=128))
        nc.sync.dma_start(xslT[:], xslT_d[:])
        nc.sync.dma_start(bg_sb[:], bg_d[:])
        nc.sync.dma_start(bo_sb[:], bo_d[:])
        nc.sync.dma_start(bgo_sb[:], bgo_d[:])
        nc.sync.dma_start(
            wgo_sb[:], wgo_d.ap().rearrange("(g p) o -> p (g o)", p=128))

        glob = persist.tile([128, N], F32)
        loc = persist.tile([128, N], BF)
        diff_bf = persist.tile([128, N], BF)
        glob_bf = persist.tile([128, N], BF)

        with ExitStack() as phAB:
            keep = phAB.enter_context(tc.tile_pool(name="keep", bufs=1))
            qT = keep.tile([128, N], BF)
            klT = keep.tile([128, N], BF)
            qpT = keep.tile([128, N], BF)
            kpT = keep.tile([128, N], BF)
            knat = keep.tile([128, N], BF)
            kpnat = keep.tile([128, N], BF)
            vaug = keep.tile([128, 2 * NCH, 65], BF)
            vlaug = keep.tile([128, 2 * NCH, 65], BF)
            vnat = keep.tile([128, N], BF)
            S_sb = keep.tile([128, 5, 65], F32)
            S_bf = keep.tile([128, 5, 65], BF)

            with ExitStack() as phA:
                trans = phA.enter_context(tc.tile_pool(name="trans", bufs=1))
                ps_tr = phA.enter_context(
                    tc.tile_pool(name="ps_tr", bufs=3, space="PSUM"))
                phX = phA.enter_context(ExitStack())
                xT_p = phX.enter_context(tc.tile_pool(name="xTp", bufs=1))
                wp_p = phX.enter_context(tc.tile_pool(name="wproj", bufs=1))
                ps_mm = phX.enter_context(
                    tc.tile_pool(name="ps_mm", bufs=1, space="PSUM"))

                # softmax(haar_scale) -> w5b = [1, sw0..sw3] broadcast down
                hs = cpool.tile([1, L], F32)
                nc.sync.dma_start(hs[:], hs_d[:])
                e4 = cpool.tile([1, L], F32)
                s1 = cpool.tile([1, 1], F32)
                nc.scalar.activation(e4[:], hs[:], AF.Exp, accum_out=s1[:])
                r1 = cpool.tile([1, 1], F32)
                nc.vector.reciprocal(r1[:], s1[:])
                w5 = cpool.tile([1, 5], BF)
                nc.vector.memset(w5[:, 0:1], 1.0)
                nc.vector.tensor_scalar_mul(w5[:, 1:5], e4[:], r1[:])
                w5bp = ps_tr.tile([128, 5], F32, tag="ptr")
                nc.tensor.matmul(w5bp[:], ones_row[:], w5[:],
                                 start=True, stop=True)
                nc.any.tensor_copy(w5b[:], w5bp[:])

                # ----- x^T arrives pre-transposed from the host -----
                xT = xT_p.tile([128, 8, N], BF)
                for k in range(8):
                    nc.sync.dma_start(xT[:, k, :], xT_d[:, k, :])
                wsb = {}
                for p in ("q", "k", "v", "kl", "vl"):
                    wsb[p] = wp_p.tile([128, 8, 128], BF, tag=f"w{p}",
                                       name=f"wsb_{p}")
                    nc.sync.dma_start(
                        wsb[p][:],
                        wT[p].ap().rearrange("(k p) m -> p k m", p=128))

                # ----- projections (k-outer so compute pipelines the DMA) ---
                kTt = trans.tile([128, N], BF)
                vTt = trans.tile([128, N], BF)
                vlTt = trans.tile([128, N], BF)
                for p, dst in (("q", qT), ("k", kTt), ("v", vTt),
                               ("kl", klT), ("vl", vlTt)):
                    accs = [ps_mm.tile([128, 512], F32, tag=f"pacc{nb}",
                                       name=f"acc_{p}_{nb}")
                            for nb in range(4)]
                    for k in range(8):
                        for nb in range(4):
                            nc.tensor.matmul(
                                accs[nb][:], wsb[p][:, k, :],
                                xT[:, k, 512 * nb:512 * (nb + 1)],
                                start=(k == 0), stop=(k == 7))
                    for nb in range(4):
                        nc.any.tensor_copy(dst[:, 512 * nb:512 * (nb + 1)],
                                           accs[nb][:])

                phX.close()
                tmp_p = phA.enter_context(tc.tile_pool(name="phitmp", bufs=2))

                # ----- phi(q), phi(k) -----
                def phi_big(dst, src):
                    tmp = tmp_p.tile([128, N], BF, tag="phitmp")
                    nc.vector.tensor_scalar_min(tmp[:], src[:], 0.0)
                    nc.scalar.activation(dst[:], tmp[:], AF.Exp)
                    nc.vector.scalar_tensor_tensor(
                        dst[:], src[:], 0.0, dst[:], op0=OP.max, op1=OP.add)

                phi_big(qpT, qT)
                phi_big(kpT, kTt)

                # ----- natural layouts via PE transpose -----
                nc.vector.memset(vaug[:, :, 64:65], 1.0)
                nc.vector.memset(vlaug[:, :, 64:65], 1.0)
                for i in range(NCH):
                    sl = slice(CH * i, CH * (i + 1))
                    pt = ps_tr.tile([128, 128], BF, tag="ptr")
                    nc.tensor.transpose(pt[:], kTt[:, sl], ident[:])
                    nc.any.tensor_copy(knat[:, sl], pt[:])
                    for src, dst in ((vTt, vaug), (vlTt, vlaug)):
                        pt = ps_tr.tile([128, 128], BF, tag="ptr")
                        nc.tensor.transpose(pt[:], src[:, sl], ident[:])
                        for h in range(2):
                            nc.any.tensor_copy(dst[:, 2 * i + h, 0:64],
                                               pt[:, 64 * h:64 * h + 64])
                        if dst is vaug:
                            nc.any.tensor_copy(vnat[:, sl], pt[:])
                # phi commutes with transpose: kpnat = phi(knat)
                phi_big(kpnat, knat)

            # ----- chunk-major recurrence + local attention -----
            bm_p = phAB.enter_context(tc.tile_pool(name="bm", bufs=3))
            lvl_p = phAB.enter_context(tc.tile_pool(name="lvl", bufs=6))
            atm_p = phAB.enter_context(tc.tile_pool(name="atm", bufs=3))
            tin_p = phAB.enter_context(tc.tile_pool(name="tiny", bufs=4))
            ps_A = phAB.enter_context(
                tc.tile_pool(name="ps_A", bufs=2, space="PSUM"))
            ps_O = phAB.enter_context(
                tc.tile_pool(name="ps_O", bufs=2, space="PSUM"))
            ps_Sd = phAB.enter_context(
                tc.tile_pool(name="ps_Sd", bufs=1, space="PSUM"))
            ps_Lo = phAB.enter_context(
                tc.tile_pool(name="ps_Lo", bufs=1, space="PSUM"))
            ps_h = phAB.enter_context(
                tc.tile_pool(name="ps_h", bufs=2, space="PSUM"))

            def phi_small(psrc, tag):
                tmp = tin_p.tile([128, 128], BF, tag="phs")
                nc.vector.tensor_scalar_min(tmp[:], psrc[:], 0.0)
                dst = lvl_p.tile([128, 128], BF, tag=tag)
                nc.scalar.activation(dst[:], tmp[:], AF.Exp)
                nc.vector.scalar_tensor_tensor(
                    dst[:], psrc[:], 0.0, dst[:], op0=OP.max, op1=OP.add)
                return dst

            for i in range(NCH):
                sl = slice(CH * i, CH * (i + 1))
                kplT, kplN, vlvA = [], [], []
                for lv in range(L):
                    bmk_ps = ps_h.tile([128, 128], F32, tag="psh")
                    nc.tensor.matmul(bmk_ps[:], knat[:, sl], Ml_sb[:, lv, :],
                                     start=True, stop=True)
                    bmk = bm_p.tile([128, 128], BF, tag="bmk")
                    nc.any.tensor_copy(bmk[:], bmk_ps[:])
                    bmv_ps = ps_h.tile([128, 128], F32, tag="psh")
                    nc.tensor.matmul(bmv_ps[:], vnat[:, sl],
                                     Ml_sb[:, lv, :], start=True, stop=True)
                    bmv = bm_p.tile([128, 128], BF, tag="bmv")
                    nc.any.tensor_copy(bmv[:], bmv_ps[:])

                    kt_ps = ps_h.tile([128, 128], F32, tag="psh")
                    nc.tensor.matmul(kt_ps[:], bdWkT[:, lv, :], bmk[:],
                                     start=True, stop=True)
                    kplT.append(phi_small(kt_ps, "kplT"))
                    kn_ps = ps_h.tile([128, 128], F32, tag="psh")
                    nc.tensor.matmul(kn_ps[:], bmk[:], bdWkT[:, lv, :],
                                     start=True, stop=True)
                    kplN.append(phi_small(kn_ps, "kplN"))
                    vn_ps = ps_h.tile([128, 128], F32, tag="psh")
                    nc.tensor.matmul(vn_ps[:], bmv[:], bdWvT[:, lv, :],
                                     start=True, stop=True)
                    va = lvl_p.tile([128, 2, 65], BF, tag="vlv")
                    nc.vector.memset(va[:, :, 64:65], 1.0)
                    # fold the softmax(haar_scale) level weight into v here
                    for h in range(2):
                        nc.scalar.mul(va[:, h, 0:64],
                                      vn_ps[:, 64 * h:64 * h + 64],
                                      w5b[:, lv + 1:lv + 2])
                    vlvA.append(va)

                psSd = ps_Sd.tile([128, 5, 65], F32, tag="psSd")
                for h in range(2):
                    hp = slice(64 * h, 64 * h + 64)
                    psO = ps_O.tile([128, 5, 65], F32, tag="psO")
                    for lv in range(5):
                        if lv == 0:
                            kpT_l = kpT[hp, sl]
                            va_l = vaug[:, 2 * i + h, :]
                        else:
                            kpT_l = kplT[lv - 1][hp, :]
                            va_l = vlvA[lv - 1][:, h, :]
                        psA = ps_A.tile([128, 128], F32, tag="psA")
                        nc.tensor.matmul(psA[:], kpT_l, qpT[hp, sl],
                                         start=True, stop=True)
                        atm = atm_p.tile([128, 128], BF, tag="atm")
                        nc.vector.tensor_mul(atm[:], psA[:], maskT[:])
                        nc.tensor.matmul(psO[:, lv, :], atm[:], va_l,
                                         start=True, stop=(i == 0))
                        if i > 0:
                            nc.tensor.matmul(psO[:, lv, :], qpT[hp, sl],
                                             S_bf[hp, lv, :],
                                             start=False, stop=True)
                    dmax = tin_p.tile([128, 5], F32, tag="dmax")
                    nc.vector.tensor_scalar_max(dmax[:], psO[:, :, 64], EPS)
                    rec = tin_p.tile([128, 5], F32, tag="rec")
                    nc.vector.reciprocal(rec[:], dmax[:])
                    gsl = glob[:, CH * i + 64 * h:CH * i + 64 * h + 64]
                    nc.vector.tensor_scalar_mul(gsl, psO[:, 0, 0:64],
                                                rec[:, 0:1])
                    for lv in range(1, 5):
                        nc.vector.scalar_tensor_tensor(
                            gsl, psO[:, lv, 0:64], rec[:, lv:lv + 1], gsl,
                            op0=OP.mult, op1=OP.add)
                    for lv in range(5):
                        if lv == 0:
                            kn_l = kpnat[:, CH * i + 64 * h:CH * i + 64 * h + 64]
                            va_l = vaug[:, 2 * i + h, :]
                        else:
                            kn_l = kplN[lv - 1][:, hp]
                            va_l = vlvA[lv - 1][:, h, :]
                        nc.tensor.matmul(psSd[hp, lv, :], kn_l, va_l,
                                         start=True, stop=True)
                if i == 0:
                    nc.vector.tensor_copy(S_sb[:], psSd[:])
                else:
                    nc.vector.tensor_add(S_sb[:], S_sb[:], psSd[:])
                if i < NCH - 1:
                    nc.any.tensor_copy(S_bf[:], S_sb[:])

                for h in range(2):
                    hp = slice(64 * h, 64 * h + 64)
                    psL = ps_A.tile([128, 256], F32, tag="psA")
                    if i > 0:
                        nc.tensor.matmul(psL[:, 0:128],
                                         klT[hp, CH * (i - 1):CH * i],
                                         qT[hp, sl], start=True, stop=True)
                    nc.tensor.matmul(psL[:, 128:256], klT[hp, sl], qT[hp, sl],
                                     start=True, stop=True)
                    P = atm_p.tile([128, 256], BF, tag="P")
                    if i > 0:
                        nc.scalar.activation(P[:], psL[:], AF.Exp, scale=0.125)
                        nc.vector.tensor_mul(P[:], P[:], lmask[:])
                    else:
                        nc.scalar.activation(P[:, 128:256], psL[:, 128:256],
                                             AF.Exp, scale=0.125)
                        nc.vector.tensor_mul(P[:, 128:256], P[:, 128:256],
                                             lmask[:, 128:256])
                    psLo = ps_Lo.tile([128, 65], F32, tag="psLo")
                    if i > 0:
                        nc.tensor.matmul(psLo[:], P[:, 0:128],
                                         vlaug[:, 2 * (i - 1) + h, :],
                                         start=True, stop=False)
                    nc.tensor.matmul(psLo[:], P[:, 128:256],
                                     vlaug[:, 2 * i + h, :],
                                     start=(i == 0), stop=True)
                    dm = tin_p.tile([128, 1], F32, tag="dm")
                    nc.vector.tensor_scalar_max(dm[:], psLo[:, 64:65], 1e-30)
                    rl = tin_p.tile([128, 1], F32, tag="rl")
                    nc.vector.reciprocal(rl[:], dm[:])
                    nc.scalar.mul(loc[:, CH * i + 64 * h:CH * i + 64 * h + 64],
                                  psLo[:, 0:64], rl[:])

            nc.vector.tensor_sub(diff_bf[:], loc[:], glob[:])
            nc.any.tensor_copy(glob_bf[:], glob[:])
            for src, t in ((diff_bf, 0), (glob_bf, 1)):
                for c2 in range(2):
                    nc.sync.dma_start(
                        a2a_in.ap()[:, t, c2].rearrange("j p m -> p j m"),
                        src[:].rearrange("p (j c m) -> p j c m",
                                         c=2, m=128)[:, :, c2, :])

        nc.gpsimd.collective_compute(
            "AllToAll", OP.bypass,
            ins=[a2a_in.ap().opt()], outs=[a2a_out.ap().opt()],
            replica_groups=[list(range(NCORES))])

        # ---------- sequence-parallel tail ----------
        with ExitStack() as phC:
            tl = phC.enter_context(tc.tile_pool(name="tail", bufs=1))
            ps_tr2 = phC.enter_context(
                tc.tile_pool(name="ps_tr2", bufs=2, space="PSUM"))
            ps_g = phC.enter_context(
                tc.tile_pool(name="ps_g", bufs=1, space="PSUM"))

            diff_g = tl.tile([128, 2, DM], BF)
            glob_g = tl.tile([128, 2, DM], BF)
            for t2 in range(2):
                nc.sync.dma_start(
                    diff_g[:, t2, :].rearrange("p (s m) -> p s m", s=8),
                    a2a_out.ap()[:, 0, t2].rearrange("s p m -> p s m"))
                nc.sync.dma_start(
                    glob_g[:, t2, :].rearrange("p (s m) -> p s m", s=8),
                    a2a_out.ap()[:, 1, t2].rearrange("s p m -> p s m"))

            diffT = tl.tile([128, 8, 256], BF)
            for t2 in range(2):
                for k in range(8):
                    pt2 = ps_tr2.tile([128, 128], BF, tag="ptr2")
                    nc.tensor.transpose(
                        pt2[:], diff_g[:, t2, 128 * k:128 * (k + 1)], ident[:])
                    nc.any.tensor_copy(diffT[:, k, 128 * t2:128 * (t2 + 1)],
                                       pt2[:])

            gh = tl.tile([128, 2, DM], BF)
            psG = []
            for j in range(4):
                psG_t = ps_g.tile([128, 512], F32, tag=f"psG{j}")
                psG.append(psG_t)
            for kc in range(16):
                for t2 in range(2):
                    lhs = (xslT[:, kc, 128 * t2:128 * (t2 + 1)] if kc < 8
                           else diffT[:, kc - 8, 128 * t2:128 * (t2 + 1)])
                    for g2 in range(2):
                        nc.tensor.matmul(
                            psG[2 * t2 + g2][:], lhs,
                            wg_sb[:, kc, 512 * g2:512 * (g2 + 1)],
                            start=(kc == 0), stop=False)
            for t2 in range(2):
                for g2 in range(2):
                    nc.tensor.matmul(
                        psG[2 * t2 + g2][:], ones_row[:],
                        bg_sb[:, 512 * g2:512 * (g2 + 1)],
                        start=False, stop=True)
                    nc.scalar.activation(
                        gh[:, t2, 512 * g2:512 * (g2 + 1)],
                        psG[2 * t2 + g2][:], AF.Silu)

            ghT = tl.tile([128, 8, 256], BF)
            for t2 in range(2):
                for k in range(8):
                    pt = ps_tr2.tile([128, 128], BF, tag="ptr2")
                    nc.tensor.transpose(
                        pt[:], gh[:, t2, 128 * k:128 * (k + 1)], ident[:])
                    nc.any.tensor_copy(ghT[:, k, 128 * t2:128 * (t2 + 1)],
                                       pt[:])

            psAl = ps_tr2.tile([128, 2], F32, tag="psAl")
            for t2 in range(2):
                for gc in range(8):
                    nc.tensor.matmul(psAl[:, t2:t2 + 1],
                                     ghT[:, gc, 128 * t2:128 * (t2 + 1)],
                                     wgo_sb[:, gc:gc + 1],
                                     start=(gc == 0), stop=False)
                nc.tensor.matmul(psAl[:, t2:t2 + 1], ones_row[:], bgo_sb[:],
                                 start=False, stop=True)
            alpha = tl.tile([128, 2], F32)
            nc.scalar.activation(alpha[:], psAl[:], AF.Sigmoid)

            mx = tl.tile([128, 2, DM], BF)
            for t2 in range(2):
                nc.vector.scalar_tensor_tensor(
                    mx[:, t2, :], diff_g[:, t2, :], alpha[:, t2:t2 + 1],
                    glob_g[:, t2, :], op0=OP.mult, op1=OP.add)
            mxT = tl.tile([128, 8, 256], BF)
            for t2 in range(2):
                for k in range(8):
                    pt = ps_tr2.tile([128, 128], BF, tag="ptr2")
                    nc.tensor.transpose(
                        pt[:], mx[:, t2, 128 * k:128 * (k + 1)], ident[:])
                    nc.any.tensor_copy(mxT[:, k, 128 * t2:128 * (t2 + 1)],
                                       pt[:])

            out_sb = tl.tile([128, 2, DM], F32)
            psF = []
            for j in range(4):
                psF_t = ps_g.tile([128, 512], F32, tag=f"psG{j}")
                psF.append(psF_t)
            for kc in range(8):
                for t2 in range(2):
                    for o2 in range(2):
                        nc.tensor.matmul(
                            psF[2 * t2 + o2][:],
                            mxT[:, kc, 128 * t2:128 * (t2 + 1)],
                            wo_sb[:, kc, 512 * o2:512 * (o2 + 1)],
                            start=(kc == 0), stop=False)
            for t2 in range(2):
                for o2 in range(2):
                    nc.tensor.matmul(
                        psF[2 * t2 + o2][:], ones_row[:],
                        bo_sb[:, 512 * o2:512 * (o2 + 1)],
                        start=False, stop=True)
                    nc.any.tensor_copy(out_sb[:, t2, 512 * o2:512 * (o2 + 1)],
                                       psF[2 * t2 + o2][:])

            nc.sync.dma_start(
                out_d.ap().rearrange("(a b) c -> b a c", b=128), out_sb[:])

    nc.compile()
    return nc


def _bf(a):
    return np.asarray(a, dtype=np.float32).astype(ml_dtypes.bfloat16)


def _prep_in_maps(x, Wq, Wk, Wv, Wkl, Wvl, haar_Wk, haar_Wv, haar_scale,
                  Wg, bg, Wgo, bgo, Wo, bo):
    ident, maskT, lmask, Ml = _host_constants()
    x2 = np.ascontiguousarray(np.asarray(x, dtype=np.float32).reshape(N, DM))
    # xT[p, k, n] = x[n, 128k + p]
    xT = np.ascontiguousarray(
        x2.T.reshape(8, 128, N).transpose(1, 0, 2)).astype(ml_dtypes.bfloat16)
    bdWkT = np.stack([_blockdiag2(np.asarray(haar_Wk[lv], dtype=np.float32).T)
                      for lv in range(L)])
    bdWvT = np.stack([_blockdiag2(np.asarray(haar_Wv[lv], dtype=np.float32).T)
                      for lv in range(L)])
    wgT = np.ascontiguousarray(np.asarray(Wg, dtype=np.float32).T)
    woT = np.ascontiguousarray(np.asarray(Wo, dtype=np.float32).T)
    wgo = np.ascontiguousarray(
        np.asarray(Wgo, dtype=np.float32).reshape(1, DM).T)
    shared = {
        "xT": xT, "bdWkT": _bf(bdWkT), "bdWvT": _bf(bdWvT), "Ml": _bf(Ml),
        "maskT": _bf(maskT), "lmask": _bf(lmask), "ident": _bf(ident),
        "wgT": _bf(wgT), "woT": _bf(woT), "wgo": _bf(wgo),
        "bg": _bf(np.asarray(bg, dtype=np.float32).reshape(1, DM)),
        "bo": _bf(np.asarray(bo, dtype=np.float32).reshape(1, DM)),
        "bgo": _bf(np.asarray(bgo, dtype=np.float32).reshape(1, 1)),
        "hscale": np.asarray(haar_scale, dtype=np.float32).reshape(1, L),
    }
    in_maps = []
    for c in range(NCORES):
        sc = slice(128 * c, 128 * (c + 1))
        m = dict(shared)
        for nm, W in (("wqT", Wq), ("wkT", Wk), ("wvT", Wv),
                      ("wklT", Wkl), ("wvlT", Wvl)):
            m[nm] = _bf(np.ascontiguousarray(
                np.asarray(W, dtype=np.float32)[sc, :].T))
        # xslT[p, k, r] = x[256c + r, 128k + p]
        xsl = x2[NSL * c:NSL * (c + 1), :]
        m["xslT"] = np.ascontiguousarray(
            xsl.T.reshape(8, 128, NSL).transpose(1, 0, 2)).astype(
                ml_dtypes.bfloat16)
        in_maps.append(m)
    return in_maps


def kernel_run(inputs, trace=False):
    if "nc" not in _CACHE:
        _CACHE["nc"] = _build_nc()
    nc = _CACHE["nc"]
    in_maps = _prep_in_maps(**inputs)
    res = run_bass_kernel_spmd(nc, in_maps, list(range(NCORES)), trace=trace)
    out = np.concatenate([res.results[c]["out"] for c in range(NCORES)],
                         axis=0)
    return out.reshape(1, N, DM).astype(np.float32), res


def kernel(**inputs):
    out, _ = kernel_run(inputs, trace=False)
    return out


# revision 21
# speedup vs baseline: 2.4732x; 1.1500x over previous
"""HALE attention (local windowed SDPA + chunked causal linear attention with
multiscale Haar context + adaptive gate) on 8 Trainium2 NeuronCores.

Sharding (B=1, so no batch DP):
  - 16 heads -> 2 heads per core (tensor-parallel over heads), packed into the
    128-partition dim for the q/k/v/local projections, the chunked
    linear-attention recurrence, and the 4 Haar-level recurrences.
  - Tail (gate + out_proj) is sequence-parallel with an interleaved chunk
    assignment: core j owns chunks j and 8+j. Two AllToAlls redistribute the
    per-head outputs (diff=local-glob, glob): the first fires after chunk 7
    (hidden under chunks 8-15), the second after chunk 15 (hidden under the
    first tail half). Host restitches the rows.

bf16 everywhere on the matmul path (PSUM accumulation stays fp32; the
normalizer reciprocals and the running linear-attention state stay fp32).
x^T and the per-core x-slice^T are pre-transposed on the host. The Haar level
tensors (block means, level projections, phi) are computed once in a batched
pre-pass over all chunks, not inside the recurrence loop.
"""

import numpy as np
import ml_dtypes
from contextlib import ExitStack

import concourse.bass as bass
import concourse.bacc as bacc
import concourse.tile as tile
import concourse.mybir as mybir
from concourse.bass_utils import run_bass_kernel_spmd

F32 = mybir.dt.float32
BF = mybir.dt.bfloat16
AF = mybir.ActivationFunctionType
OP = mybir.AluOpType

NCORES = 8
N = 2048
DM = 1024
H = 16
DH = 64
L = 4
CH = 128
NCH = N // CH
WIN = 64
NSL = N // NCORES
EPS = 1e-6

_CACHE = {}


def _host_constants():
    ident = np.eye(128, dtype=np.float32)
    ck = np.arange(CH)[:, None]
    cq = np.arange(CH)[None, :]
    maskT = (ck <= cq).astype(np.float32)
    prev = (ck >= cq + WIN + 1).astype(np.float32)
    cur = ((ck <= cq) & (ck >= cq - (WIN - 1))).astype(np.float32)
    lmask = np.concatenate([prev, cur], axis=1)
    Ml = np.zeros((L, CH, CH), dtype=np.float32)
    for lv in range(L):
        b = 2 ** (lv + 1)
        m = np.arange(CH)[:, None]
        n = np.arange(CH)[None, :]
        Ml[lv] = np.where(((m // b) == (n // b)) & (m <= n),
                          1.0 / (n % b + 1.0), 0.0)
    return ident, maskT, lmask, Ml


def _blockdiag2(a):
    z = np.zeros((128, 128), dtype=np.float32)
    z[:64, :64] = a
    z[64:, 64:] = a
    return z


def _build_nc():
    nc = bacc.Bacc("TRN2", target_bir_lowering=False, debug=False,
                   num_devices=NCORES)

    xT_d = nc.dram_tensor("xT", [128, 8, N], BF, kind="ExternalInput")
    xslT_d = nc.dram_tensor("xslT", [128, 8, NSL], BF, kind="ExternalInput")
    wT = {p: nc.dram_tensor(f"w{p}T", [128, 8, 128], BF, kind="ExternalInput")
          for p in ("q", "k", "v", "kl", "vl")}
    bdWkT_d = nc.dram_tensor("bdWkT", [128, L, 128], BF, kind="ExternalInput")
    bdWvT_d = nc.dram_tensor("bdWvT", [128, L, 128], BF, kind="ExternalInput")
    Ml_d = nc.dram_tensor("Ml", [128, L, 128], BF, kind="ExternalInput")
    maskT_d = nc.dram_tensor("maskT", [128, 128], BF, kind="ExternalInput")
    lmask_d = nc.dram_tensor("lmask", [128, 256], BF, kind="ExternalInput")
    ident_d = nc.dram_tensor("ident", [128, 128], BF, kind="ExternalInput")
    wgT_d = nc.dram_tensor("wgT", [128, 16, DM], BF, kind="ExternalInput")
    woT_d = nc.dram_tensor("woT", [128, 8, DM], BF, kind="ExternalInput")
    wgo_d = nc.dram_tensor("wgo", [128, 8], BF, kind="ExternalInput")
    bg_d = nc.dram_tensor("bg", [1, DM], BF, kind="ExternalInput")
    bo_d = nc.dram_tensor("bo", [1, DM], BF, kind="ExternalInput")
    bgo_d = nc.dram_tensor("bgo", [1, 1], BF, kind="ExternalInput")
    hs_d = nc.dram_tensor("hscale", [1, L], F32, kind="ExternalInput")
    out_d = nc.dram_tensor("out", [NSL, DM], F32, kind="ExternalOutput")

    # [dest, tensor(diff,glob), 128, 128]; a2a1 = chunks 0-7, a2a2 = 8-15
    a2a1_in = nc.dram_tensor("a2a1_in", [NCORES, 2, 128, 128], BF)
    a2a1_out = nc.dram_tensor("a2a1_out", [NCORES, 2, 128, 128], BF)
    a2a2_in = nc.dram_tensor("a2a2_in", [NCORES, 2, 128, 128], BF)
    a2a2_out = nc.dram_tensor("a2a2_out", [NCORES, 2, 128, 128], BF)

    with tile.TileContext(nc) as tc, ExitStack() as root:
        cpool = root.enter_context(tc.tile_pool(name="consts", bufs=1))
        persist = root.enter_context(tc.tile_pool(name="persist", bufs=1))

        ident = cpool.tile([128, 128], BF)
        maskT = cpool.tile([128, 128], BF)
        lmask = cpool.tile([128, 256], BF)
        Ml_sb = cpool.tile([128, L, 128], BF)
        bdWkT = cpool.tile([128, L, 128], BF)
        bdWvT = cpool.tile([128, L, 128], BF)
        ones_row = cpool.tile([1, 128], BF)
        w5b = cpool.tile([128, 5], F32)
        hs = cpool.tile([1, L], F32)

        glob = persist.tile([128, N], F32)
        loc = persist.tile([128, N], BF)
        diff_bf = persist.tile([128, N], BF)
        glob_bf = persist.tile([128, N], BF)
        wg_sb = persist.tile([128, 16, DM], BF)
        wo_sb = persist.tile([128, 8, DM], BF)
        xslT = persist.tile([128, 8, NSL], BF)
        bg_sb = cpool.tile([1, DM], BF)
        bo_sb = cpool.tile([1, DM], BF)
        bgo_sb = cpool.tile([1, 1], BF)
        wgo_sb = cpool.tile([128, 8], BF)

        with ExitStack() as phAB:
            keep = phAB.enter_context(tc.tile_pool(name="keep", bufs=1))
            qT = keep.tile([128, N], BF)
            klT = keep.tile([128, N], BF)
            qpT = keep.tile([128, N], BF)
            kpT = keep.tile([128, N], BF)
            knat = keep.tile([128, N], BF)
            kpnat = keep.tile([128, N], BF)
            vaug = keep.tile([128, 2 * NCH, 65], BF)
            vlaug = keep.tile([128, 2 * NCH, 65], BF)
            vnat = keep.tile([128, N], BF)
            S_sb = keep.tile([128, 5, 65], F32)
            S_bf = keep.tile([128, 5, 65], BF)

            with ExitStack() as phA:
                trans = phA.enter_context(tc.tile_pool(name="trans", bufs=1))
                phX = phA.enter_context(ExitStack())
                xT_p = phX.enter_context(tc.tile_pool(name="xTp", bufs=1))
                wp_p = phX.enter_context(tc.tile_pool(name="wproj", bufs=1))
                ps_mm = phX.enter_context(
                    tc.tile_pool(name="ps_mm", bufs=1, space="PSUM"))

                # projection inputs first; all DMAs are contiguous
                # per-partition copies (host pre-arranged)
                xT = xT_p.tile([128, 8, N], BF)
                wsb = {}
                for k in range(2):
                    nc.sync.dma_start(xT[:, k, :], xT_d[:, k, :])
                for p in ("q", "k", "v", "kl", "vl"):
                    wsb[p] = wp_p.tile([128, 8, 128], BF, tag=f"w{p}",
                                       name=f"wsb_{p}")
                    nc.sync.dma_start(wsb[p][:], wT[p].ap())
                for k in range(2, 8):
                    nc.sync.dma_start(xT[:, k, :], xT_d[:, k, :])

                nc.sync.dma_start(ident[:], ident_d[:])
                nc.sync.dma_start(maskT[:], maskT_d[:])
                nc.sync.dma_start(lmask[:], lmask_d[:])
                nc.sync.dma_start(Ml_sb[:], Ml_d.ap())
                nc.sync.dma_start(bdWkT[:], bdWkT_d.ap())
                nc.sync.dma_start(bdWvT[:], bdWvT_d.ap())
                nc.sync.dma_start(hs[:], hs_d[:])
                nc.vector.memset(ones_row[:], 1.0)

                # tail weights prefetch (have ~250us of slack)
                nc.sync.dma_start(wg_sb[:], wgT_d.ap())
                nc.sync.dma_start(wo_sb[:], woT_d.ap())
                nc.sync.dma_start(xslT[:], xslT_d[:])
                nc.sync.dma_start(bg_sb[:], bg_d[:])
                nc.sync.dma_start(bo_sb[:], bo_d[:])
                nc.sync.dma_start(bgo_sb[:], bgo_d[:])
                nc.sync.dma_start(wgo_sb[:], wgo_d.ap())

                # softmax(haar_scale) -> w5b[:, 1:5], broadcast down columns
                e4 = cpool.tile([1, L], F32)
                s1 = cpool.tile([1, 1], F32)
                nc.scalar.activation(e4[:], hs[:], AF.Exp, accum_out=s1[:])
                r1 = cpool.tile([1, 1], F32)
                nc.vector.reciprocal(r1[:], s1[:])
                w5 = cpool.tile([1, 5], BF)
                nc.vector.memset(w5[:, 0:1], 1.0)
                nc.vector.tensor_scalar_mul(w5[:, 1:5], e4[:], r1[:])
                w5bp = ps_mm.tile([128, 5], F32, tag="w5bp")
                nc.tensor.matmul(w5bp[:], ones_row[:], w5[:],
                                 start=True, stop=True)
                nc.vector.tensor_copy(w5b[:], w5bp[:])

                # ----- projections (k-outer so compute pipelines the DMA) ---
                kTt = trans.tile([128, N], BF)
                vTt = trans.tile([128, N], BF)
                vlTt = trans.tile([128, N], BF)
                for p, dst in (("q", qT), ("k", kTt), ("v", vTt),
                               ("kl", klT), ("vl", vlTt)):
                    accs = [ps_mm.tile([128, 512], F32, tag=f"pacc{nb}",
                                       name=f"acc_{p}_{nb}")
                            for nb in range(4)]
                    for k in range(8):
                        for nb in range(4):
                            nc.tensor.matmul(
                                accs[nb][:], wsb[p][:, k, :],
                                xT[:, k, 512 * nb:512 * (nb + 1)],
                                start=(k == 0), stop=(k == 7))
                    for nb in range(4):
                        nc.any.tensor_copy(dst[:, 512 * nb:512 * (nb + 1)],
                                           accs[nb][:])

                phX.close()
                # allocated only now: must not coexist with xT's 32KB
                kplT_all = keep.tile([128, L, N], BF)
                kplN_all = keep.tile([128, L, N], BF)
                va_all = keep.tile([128, L, 2 * NCH, 65], BF)

                # ----- phi(q), phi(k) + natural layouts via PE transpose ----
                phT = phA.enter_context(ExitStack())
                tmp_p = phT.enter_context(tc.tile_pool(name="phitmp", bufs=2))
                ps_tr = phT.enter_context(
                    tc.tile_pool(name="ps_tr", bufs=3, space="PSUM"))

                def phi_big(dst, src):
                    tmp = tmp_p.tile([128, N], BF, tag="phitmp")
                    nc.vector.tensor_scalar_min(tmp[:], src[:], 0.0)
                    nc.scalar.activation(dst[:], tmp[:], AF.Exp)
                    nc.vector.scalar_tensor_tensor(
                        dst[:], src[:], 0.0, dst[:], op0=OP.max, op1=OP.add)

                phi_big(qpT, qT)
                phi_big(kpT, kTt)

                nc.vector.memset(vaug[:, :, 64:65], 1.0)
                nc.vector.memset(vlaug[:, :, 64:65], 1.0)
                for i in range(NCH):
                    sl = slice(CH * i, CH * (i + 1))
                    pt = ps_tr.tile([128, 128], BF, tag="ptr")
                    nc.tensor.transpose(pt[:], kTt[:, sl], ident[:])
                    nc.any.tensor_copy(knat[:, sl], pt[:])
                    for src, dst in ((vTt, vaug), (vlTt, vlaug)):
                        pt = ps_tr.tile([128, 128], BF, tag="ptr")
                        nc.tensor.transpose(pt[:], src[:, sl], ident[:])
                        for h in range(2):
                            nc.any.tensor_copy(dst[:, 2 * i + h, 0:64],
                                               pt[:, 64 * h:64 * h + 64])
                        if dst is vaug:
                            nc.any.tensor_copy(vnat[:, sl], pt[:])
                # phi commutes with transpose: kpnat = phi(knat)
                phi_big(kpnat, knat)
                phT.close()

                # ----- Haar level tensors, batched across all chunks -------
                phH = phA.enter_context(ExitStack())
                bma_p = phH.enter_context(tc.tile_pool(name="bmall", bufs=2))
                tmpb_p = phH.enter_context(tc.tile_pool(name="tmpb", bufs=2))
                ps_bk = phH.enter_context(
                    tc.tile_pool(name="ps_bk", bufs=2, space="PSUM"))
                ps_bg = phH.enter_context(
                    tc.tile_pool(name="ps_bg", bufs=2, space="PSUM"))

                def phi_blk(dst, psrc):
                    tmp = tmpb_p.tile([128, 512], BF, tag="phb")
                    nc.vector.tensor_scalar_min(tmp[:], psrc, 0.0)
                    nc.scalar.activation(dst, tmp[:], AF.Exp)
                    nc.vector.scalar_tensor_tensor(
                        dst, psrc, 0.0, dst, op0=OP.max, op1=OP.add)

                for lv in range(L):
                    bmk_all = bma_p.tile([128, N], BF, tag="bmk_all")
                    bmv_all = bma_p.tile([128, N], BF, tag="bmv_all")
                    for g in range(4):
                        gs = slice(512 * g, 512 * (g + 1))
                        pk = ps_bk.tile([128, 4, 128], F32, tag="pk")
                        pv = ps_bk.tile([128, 4, 128], F32, tag="pv")
                        for j in range(4):
                            i = 4 * g + j
                            sl = slice(CH * i, CH * (i + 1))
                            nc.tensor.matmul(pk[:, j, :], knat[:, sl],
                                             Ml_sb[:, lv, :],
                                             start=True, stop=True)
                            nc.tensor.matmul(pv[:, j, :], vnat[:, sl],
                                             Ml_sb[:, lv, :],
                                             start=True, stop=True)
                        nc.any.tensor_copy(
                            bmk_all[:, gs],
                            pk[:].rearrange("p a b -> p (a b)"))
                        # fold softmax(haar_scale)[lv] into the v block means
                        nc.scalar.mul(
                            bmv_all[:, gs],
                            pv[:].rearrange("p a b -> p (a b)"),
                            w5b[:, lv + 1:lv + 2])
                    # k_lvl transposed orientation: bdWk^T @ bm, then phi
                    for g in range(4):
                        gs = slice(512 * g, 512 * (g + 1))
                        pT = ps_bg.tile([128, 512], F32, tag="pT")
                        nc.tensor.matmul(pT[:], bdWkT[:, lv, :],
                                         bmk_all[:, gs],
                                         start=True, stop=True)
                        phi_blk(kplT_all[:, lv, gs], pT[:])
                    # k_lvl natural orientation: bm @ bdWk, then phi
                    for g in range(4):
                        gs = slice(512 * g, 512 * (g + 1))
                        pN = ps_bk.tile([128, 4, 128], F32, tag="pk")
                        for j in range(4):
                            i = 4 * g + j
                            sl = slice(CH * i, CH * (i + 1))
                            nc.tensor.matmul(pN[:, j, :], bmk_all[:, sl],
                                             bdWkT[:, lv, :],
                                             start=True, stop=True)
                        phi_blk(kplN_all[:, lv, gs],
                                pN[:].rearrange("p a b -> p (a b)"))
                    # v_lvl natural (already w-scaled via bmv_all)
                    nc.vector.memset(va_all[:, lv, :, 64:65], 1.0)
                    for g in range(4):
                        pV = ps_bk.tile([128, 4, 128], F32, tag="pv")
                        for j in range(4):
                            i = 4 * g + j
                            sl = slice(CH * i, CH * (i + 1))
                            nc.tensor.matmul(pV[:, j, :], bmv_all[:, sl],
                                             bdWvT[:, lv, :],
                                             start=True, stop=True)
                        nc.any.tensor_copy(
                            va_all[:, lv, 8 * g:8 * (g + 1), 0:64],
                            pV[:].rearrange("p a (h d) -> p (a h) d", h=2))
                phH.close()

            # ----- chunk-major recurrence + local attention -----
            atm_p = phAB.enter_context(tc.tile_pool(name="atm", bufs=3))
            tin_p = phAB.enter_context(tc.tile_pool(name="tiny", bufs=4))
            ps_A = phAB.enter_context(
                tc.tile_pool(name="ps_A", bufs=2, space="PSUM"))
            ps_O = phAB.enter_context(
                tc.tile_pool(name="ps_O", bufs=2, space="PSUM"))
            ps_Sd = phAB.enter_context(
                tc.tile_pool(name="ps_Sd", bufs=1, space="PSUM"))
            ps_Lo = phAB.enter_context(
                tc.tile_pool(name="ps_Lo", bufs=2, space="PSUM"))

            def a2a_quarter(q4, a2a_in_t):
                # chunks 4*q4 .. 4*q4+3 -> dest slices of the matching a2a
                half, jg = divmod(q4, 2)
                cs = slice(512 * q4, 512 * (q4 + 1))
                nc.vector.tensor_sub(diff_bf[:, cs], loc[:, cs], glob[:, cs])
                nc.vector.tensor_copy(glob_bf[:, cs], glob[:, cs])
                jsl = slice(4 * jg, 4 * (jg + 1))
                nc.sync.dma_start(
                    a2a_in_t.ap()[jsl, 0].rearrange("j p m -> p j m"),
                    diff_bf[:, cs].rearrange("p (j m) -> p j m", m=128))
                nc.sync.dma_start(
                    a2a_in_t.ap()[jsl, 1].rearrange("j p m -> p j m"),
                    glob_bf[:, cs].rearrange("p (j m) -> p j m", m=128))

            for i in range(NCH):
                sl = slice(CH * i, CH * (i + 1))
                psSd = ps_Sd.tile([128, 5, 65], F32, tag="psSd")
                for h in range(2):
                    hp = slice(64 * h, 64 * h + 64)
                    psO = ps_O.tile([128, 5, 65], F32, tag="psO")
                    for lv in range(5):
                        if lv == 0:
                            kpT_l = kpT[hp, sl]
                            va_l = vaug[:, 2 * i + h, :]
                        else:
                            kpT_l = kplT_all[hp, lv - 1, sl]
                            va_l = va_all[:, lv - 1, 2 * i + h, :]
                        psA = ps_A.tile([128, 128], F32, tag="psA")
                        nc.tensor.matmul(psA[:], kpT_l, qpT[hp, sl],
                                         start=True, stop=True)
                        atm = atm_p.tile([128, 128], BF, tag="atm")
                        nc.vector.tensor_mul(atm[:], psA[:], maskT[:])
                        nc.tensor.matmul(psO[:, lv, :], atm[:], va_l,
                                         start=True, stop=(i == 0))
                        if i > 0:
                            nc.tensor.matmul(psO[:, lv, :], qpT[hp, sl],
                                             S_bf[hp, lv, :],
                                             start=False, stop=True)
                    dmax = tin_p.tile([128, 5], F32, tag="dmax")
                    nc.vector.tensor_scalar_max(dmax[:], psO[:, :, 64], EPS)
                    rec = tin_p.tile([128, 5], F32, tag="rec")
                    nc.vector.reciprocal(rec[:], dmax[:])
                    gsl = glob[:, CH * i + 64 * h:CH * i + 64 * h + 64]
                    nc.vector.tensor_scalar_mul(gsl, psO[:, 0, 0:64],
                                                rec[:, 0:1])
                    for lv in range(1, 5):
                        nc.vector.scalar_tensor_tensor(
                            gsl, psO[:, lv, 0:64], rec[:, lv:lv + 1], gsl,
                            op0=OP.mult, op1=OP.add)
                    for lv in range(5):
                        if lv == 0:
                            kn_l = kpnat[:,
                                         CH * i + 64 * h:CH * i + 64 * h + 64]
                            va_l = vaug[:, 2 * i + h, :]
                        else:
                            kn_l = kplN_all[:, lv - 1,
                                            CH * i + 64 * h:CH * i + 64 * h + 64]
                            va_l = va_all[:, lv - 1, 2 * i + h, :]
                        nc.tensor.matmul(psSd[hp, lv, :], kn_l, va_l,
                                         start=True, stop=True)
                if i == 0:
                    nc.vector.tensor_copy(S_sb[:], psSd[:])
                else:
                    nc.vector.tensor_add(S_sb[:], S_sb[:], psSd[:])
                if i < NCH - 1:
                    nc.scalar.copy(S_bf[:], S_sb[:])

                for h in range(2):
                    hp = slice(64 * h, 64 * h + 64)
                    psL = ps_A.tile([128, 256], F32, tag="psA")
                    if i > 0:
                        nc.tensor.matmul(psL[:, 0:128],
                                         klT[hp, CH * (i - 1):CH * i],
                                         qT[hp, sl], start=True, stop=True)
                    nc.tensor.matmul(psL[:, 128:256], klT[hp, sl], qT[hp, sl],
                                     start=True, stop=True)
                    P = atm_p.tile([128, 256], BF, tag="P")
                    if i > 0:
                        nc.scalar.activation(P[:], psL[:], AF.Exp, scale=0.125)
                        nc.vector.tensor_mul(P[:], P[:], lmask[:])
                    else:
                        nc.scalar.activation(P[:, 128:256], psL[:, 128:256],
                                             AF.Exp, scale=0.125)
                        nc.vector.tensor_mul(P[:, 128:256], P[:, 128:256],
                                             lmask[:, 128:256])
                    psLo = ps_Lo.tile([128, 65], F32, tag="psLo")
                    if i > 0:
                        nc.tensor.matmul(psLo[:], P[:, 0:128],
                                         vlaug[:, 2 * (i - 1) + h, :],
                                         start=True, stop=False)
                    nc.tensor.matmul(psLo[:], P[:, 128:256],
                                     vlaug[:, 2 * i + h, :],
                                     start=(i == 0), stop=True)
                    dm = tin_p.tile([128, 1], F32, tag="dm")
                    nc.vector.tensor_scalar_max(dm[:], psLo[:, 64:65], 1e-30)
                    rl = tin_p.tile([128, 1], F32, tag="rl")
                    nc.vector.reciprocal(rl[:], dm[:])
                    nc.scalar.mul(loc[:, CH * i + 64 * h:CH * i + 64 * h + 64],
                                  psLo[:, 0:64], rl[:])

                if i in (3, 7):
                    a2a_quarter(i // 4, a2a1_in)
                if i == 7:
                    nc.gpsimd.collective_compute(
                        "AllToAll", OP.bypass,
                        ins=[a2a1_in.ap().opt()], outs=[a2a1_out.ap().opt()],
                        replica_groups=[list(range(NCORES))])
                if i in (11, 15):
                    a2a_quarter(i // 4, a2a2_in)

        nc.gpsimd.collective_compute(
            "AllToAll", OP.bypass,
            ins=[a2a2_in.ap().opt()], outs=[a2a2_out.ap().opt()],
            replica_groups=[list(range(NCORES))])

        # ---------- sequence-parallel tail: two interleaved halves ----------
        with ExitStack() as phC:
            tl = phC.enter_context(tc.tile_pool(name="tail", bufs=2))
            ps_tr2 = phC.enter_context(
                tc.tile_pool(name="ps_tr2", bufs=2, space="PSUM"))
            ps_g = phC.enter_context(
                tc.tile_pool(name="ps_g", bufs=1, space="PSUM"))

            for h2, a2a_out_t in ((0, a2a1_out), (1, a2a2_out)):
                diff_g = tl.tile([128, DM], BF, tag="diff_g")
                glob_g = tl.tile([128, DM], BF, tag="glob_g")
                nc.sync.dma_start(
                    diff_g[:].rearrange("p (s m) -> p s m", s=8),
                    a2a_out_t.ap()[:, 0].rearrange("s p m -> p s m"))
                nc.sync.dma_start(
                    glob_g[:].rearrange("p (s m) -> p s m", s=8),
                    a2a_out_t.ap()[:, 1].rearrange("s p m -> p s m"))

                diffT = tl.tile([128, 8, 128], BF, tag="diffT")
                globT = tl.tile([128, 8, 128], BF, tag="globT")
                for srcT, dstT in ((diff_g, diffT), (glob_g, globT)):
                    for k in range(8):
                        pt2 = ps_tr2.tile([128, 128], BF, tag="ptr2")
                        nc.tensor.transpose(
                            pt2[:], srcT[:, 128 * k:128 * (k + 1)], ident[:])
                        nc.vector.tensor_copy(dstT[:, k, :], pt2[:])

                # gate hidden: [x, diff] @ Wg^T + bg -> silu
                gh = tl.tile([128, DM], BF, tag="gh")
                psG = [ps_g.tile([128, 512], F32, tag=f"psG{g2}",
                                 name=f"psG_{h2}_{g2}") for g2 in range(2)]
                for kc in range(16):
                    lhs = (xslT[:, kc, 128 * h2:128 * (h2 + 1)] if kc < 8
                           else diffT[:, kc - 8, :])
                    for g2 in range(2):
                        nc.tensor.matmul(
                            psG[g2][:], lhs,
                            wg_sb[:, kc, 512 * g2:512 * (g2 + 1)],
                            start=(kc == 0), stop=False)
                for g2 in range(2):
                    nc.tensor.matmul(
                        psG[g2][:], ones_row[:],
                        bg_sb[:, 512 * g2:512 * (g2 + 1)],
                        start=False, stop=True)
                    nc.scalar.activation(
                        gh[:, 512 * g2:512 * (g2 + 1)],
                        psG[g2][:], AF.Silu)

                # oD = diff @ Wo^T (alpha-independent, overlaps alpha chain)
                psFD = [ps_g.tile([128, 512], F32, tag=f"psFD{o2}",
                                  name=f"psFD_{h2}_{o2}") for o2 in range(2)]
                for kc in range(8):
                    for o2 in range(2):
                        nc.tensor.matmul(
                            psFD[o2][:], diffT[:, kc, :],
                            wo_sb[:, kc, 512 * o2:512 * (o2 + 1)],
                            start=(kc == 0), stop=(kc == 7))

                ghT = tl.tile([128, 8, 128], BF, tag="ghT")
                for k in range(8):
                    pt2 = ps_tr2.tile([128, 128], BF, tag="ptr2")
                    nc.tensor.transpose(
                        pt2[:], gh[:, 128 * k:128 * (k + 1)], ident[:])
                    nc.vector.tensor_copy(ghT[:, k, :], pt2[:])

                psAl = ps_tr2.tile([128, 1], F32, tag="psAl", bufs=1)
                for gc in range(8):
                    nc.tensor.matmul(psAl[:], ghT[:, gc, :],
                                     wgo_sb[:, gc:gc + 1],
                                     start=(gc == 0), stop=False)
                nc.tensor.matmul(psAl[:], ones_row[:], bgo_sb[:],
                                 start=False, stop=True)
                alpha = tl.tile([128, 1], F32, tag="alpha")
                nc.scalar.activation(alpha[:], psAl[:], AF.Sigmoid)

                # oG = glob @ Wo^T + bo, reusing the gate's PSUM banks
                psFG = [ps_g.tile([128, 512], F32, tag=f"psG{o2}",
                                  name=f"psFG_{h2}_{o2}") for o2 in range(2)]
                for kc in range(8):
                    for o2 in range(2):
                        nc.tensor.matmul(
                            psFG[o2][:], globT[:, kc, :],
                            wo_sb[:, kc, 512 * o2:512 * (o2 + 1)],
                            start=(kc == 0), stop=False)
                for o2 in range(2):
                    nc.tensor.matmul(
                        psFG[o2][:], ones_row[:],
                        bo_sb[:, 512 * o2:512 * (o2 + 1)],
                        start=False, stop=True)

                # out = alpha * oD + oG (oG staged via SBUF: the DVE
                # cannot read two PSUM operands in one op)
                oG_sb = tl.tile([128, DM], F32, tag="oG_sb")
                out_sb = tl.tile([128, DM], F32, tag="out_sb")
                for o2 in range(2):
                    nc.scalar.copy(oG_sb[:, 512 * o2:512 * (o2 + 1)],
                                   psFG[o2][:])
                for o2 in range(2):
                    nc.vector.scalar_tensor_tensor(
                        out_sb[:, 512 * o2:512 * (o2 + 1)],
                        psFD[o2][:], alpha[:],
                        oG_sb[:, 512 * o2:512 * (o2 + 1)],
                        op0=OP.mult, op1=OP.add)

                nc.sync.dma_start(out_d.ap()[128 * h2:128 * (h2 + 1), :],
                                  out_sb[:])

    nc.compile()
    return nc


def _bf(a):
    return np.asarray(a, dtype=np.float32).astype(ml_dtypes.bfloat16)


def _pkm(a, nk):
    # [nk*128, m] -> [128, nk, m] with row = 128k + p
    m = a.shape[1]
    return np.ascontiguousarray(
        a.reshape(nk, 128, m).transpose(1, 0, 2)).astype(ml_dtypes.bfloat16)


def _prep_in_maps(x, Wq, Wk, Wv, Wkl, Wvl, haar_Wk, haar_Wv, haar_scale,
                  Wg, bg, Wgo, bgo, Wo, bo):
    ident, maskT, lmask, Ml = _host_constants()
    x2 = np.ascontiguousarray(np.asarray(x, dtype=np.float32).reshape(N, DM))
    # xT[p, k, n] = x[n, 128k + p]
    xTf = x2.T.reshape(8, 128, N)
    xT = np.ascontiguousarray(xTf.transpose(1, 0, 2)).astype(
        ml_dtypes.bfloat16)
    bdWkT = np.stack([_blockdiag2(np.asarray(haar_Wk[lv], dtype=np.float32).T)
                      for lv in range(L)])
    bdWvT = np.stack([_blockdiag2(np.asarray(haar_Wv[lv], dtype=np.float32).T)
                      for lv in range(L)])
    wgT = np.ascontiguousarray(np.asarray(Wg, dtype=np.float32).T)
    woT = np.ascontiguousarray(np.asarray(Wo, dtype=np.float32).T)
    wgo = np.asarray(Wgo, dtype=np.float32).reshape(8, 128)
    shared = {
        "xT": xT,
        "bdWkT": _bf(bdWkT.transpose(1, 0, 2)),
        "bdWvT": _bf(bdWvT.transpose(1, 0, 2)),
        "Ml": _bf(Ml.transpose(1, 0, 2)),
        "maskT": _bf(maskT), "lmask": _bf(lmask), "ident": _bf(ident),
        "wgT": _pkm(wgT, 16), "woT": _pkm(woT, 8),
        "wgo": _bf(wgo.T),
        "bg": _bf(np.asarray(bg, dtype=np.float32).reshape(1, DM)),
        "bo": _bf(np.asarray(bo, dtype=np.float32).reshape(1, DM)),
        "bgo": _bf(np.asarray(bgo, dtype=np.float32).reshape(1, 1)),
        "hscale": np.asarray(haar_scale, dtype=np.float32).reshape(1, L),
    }
    in_maps = []
    for c in range(NCORES):
        sc = slice(128 * c, 128 * (c + 1))
        m = dict(shared)
        for nm, W in (("wqT", Wq), ("wkT", Wk), ("wvT", Wv),
                      ("wklT", Wkl), ("wvlT", Wvl)):
            m[nm] = _pkm(np.ascontiguousarray(
                np.asarray(W, dtype=np.float32)[sc, :].T), 8)
        # core c owns chunks c and 8+c: xslT[p, k, 128*half + r]
        m["xslT"] = np.ascontiguousarray(np.concatenate(
            [xTf[:, :, 128 * c:128 * (c + 1)],
             xTf[:, :, 128 * (8 + c):128 * (9 + c)]],
            axis=2).transpose(1, 0, 2)).astype(ml_dtypes.bfloat16)
        in_maps.append(m)
    return in_maps


def kernel_run(inputs, trace=False):
    if "nc" not in _CACHE:
        _CACHE["nc"] = _build_nc()
    nc = _CACHE["nc"]
    in_maps = _prep_in_maps(**inputs)
    res = run_bass_kernel_spmd(nc, in_maps, list(range(NCORES)), trace=trace)
    out = np.zeros((N, DM), dtype=np.float32)
    for c in range(NCORES):
        r = res.results[c]["out"]
        out[128 * c:128 * (c + 1)] = r[0:128]
        out[128 * (8 + c):128 * (9 + c)] = r[128:256]
    return out.reshape(1, N, DM), res


def kernel(**inputs):
    out, _ = kernel_run(inputs, trace=False)
    return out


# revision 22
# speedup vs baseline: 3.3770x; 1.3654x over previous
"""HALE attention (local windowed SDPA + chunked causal linear attention with
multiscale Haar context + adaptive gate) on 8 Trainium2 NeuronCores.

Sharding (B=1, so no batch DP):
  - 16 heads -> 2 heads per core (tensor-parallel over heads), packed into the
    128-partition dim for the q/k/v/local projections, the chunked
    linear-attention recurrence, and the 4 Haar-level recurrences.
  - Tail (gate + out_proj) is sequence-parallel with an interleaved chunk
    assignment: core j owns chunks j and 8+j. Two AllToAlls redistribute the
    per-head outputs (diff=local-glob, glob): the first fires after chunk 7
    (hidden under chunks 8-15), the second after chunk 15 (hidden under the
    first tail half). Host restitches the rows.

bf16 everywhere on the matmul path (PSUM accumulation stays fp32; the
normalizer reciprocals and the running linear-attention state stay fp32).
x^T and the per-core x-slice^T are pre-transposed on the host. The Haar level
tensors (block means, level projections, phi) are computed once in a batched
pre-pass over all chunks, not inside the recurrence loop.
"""

import numpy as np
import ml_dtypes
from contextlib import ExitStack

import concourse.bass as bass
import concourse.bacc as bacc
import concourse.tile as tile
import concourse.mybir as mybir
from concourse.bass_utils import run_bass_kernel_spmd

F32 = mybir.dt.float32
BF = mybir.dt.bfloat16
AF = mybir.ActivationFunctionType
OP = mybir.AluOpType

NCORES = 8
N = 2048
DM = 1024
H = 16
DH = 64
L = 4
CH = 128
NCH = N // CH
WIN = 64
NSL = N // NCORES
EPS = 1e-6

_CACHE = {}


def _host_constants():
    ident = np.eye(128, dtype=np.float32)
    ck = np.arange(CH)[:, None]
    cq = np.arange(CH)[None, :]
    maskT = (ck <= cq).astype(np.float32)
    prev = (ck >= cq + WIN + 1).astype(np.float32)
    cur = ((ck <= cq) & (ck >= cq - (WIN - 1))).astype(np.float32)
    lmask = np.concatenate([prev, cur], axis=1)
    Ml = np.zeros((L, CH, CH), dtype=np.float32)
    for lv in range(L):
        b = 2 ** (lv + 1)
        m = np.arange(CH)[:, None]
        n = np.arange(CH)[None, :]
        Ml[lv] = np.where(((m // b) == (n // b)) & (m <= n),
                          1.0 / (n % b + 1.0), 0.0)
    return ident, maskT, lmask, Ml


def _blockdiag2(a):
    z = np.zeros((128, 128), dtype=np.float32)
    z[:64, :64] = a
    z[64:, 64:] = a
    return z


def _build_nc():
    nc = bacc.Bacc("TRN2", target_bir_lowering=False, debug=False,
                   num_devices=NCORES)

    xT_d = nc.dram_tensor("xT", [128, 8, N], BF, kind="ExternalInput")
    xslT_d = nc.dram_tensor("xslT", [128, 8, NSL], BF, kind="ExternalInput")
    wT = {p: nc.dram_tensor(f"w{p}T", [128, 8, 128], BF, kind="ExternalInput")
          for p in ("q", "k", "v", "kl", "vl")}
    bdWkT_d = nc.dram_tensor("bdWkT", [128, L, 128], BF, kind="ExternalInput")
    bdWvT_d = nc.dram_tensor("bdWvT", [128, L, 128], BF, kind="ExternalInput")
    Ml_d = nc.dram_tensor("Ml", [128, L, 128], BF, kind="ExternalInput")
    maskT_d = nc.dram_tensor("maskT", [128, 128], BF, kind="ExternalInput")
    lmask_d = nc.dram_tensor("lmask", [128, 256], BF, kind="ExternalInput")
    ident_d = nc.dram_tensor("ident", [128, 128], BF, kind="ExternalInput")
    wgT_d = nc.dram_tensor("wgT", [128, 16, DM], BF, kind="ExternalInput")
    woT_d = nc.dram_tensor("woT", [128, 8, DM], BF, kind="ExternalInput")
    wgo_d = nc.dram_tensor("wgo", [128, 8], BF, kind="ExternalInput")
    bg_d = nc.dram_tensor("bg", [1, DM], BF, kind="ExternalInput")
    bo_d = nc.dram_tensor("bo", [1, DM], BF, kind="ExternalInput")
    bgo_d = nc.dram_tensor("bgo", [1, 1], BF, kind="ExternalInput")
    hs_d = nc.dram_tensor("hscale", [1, L], F32, kind="ExternalInput")
    out_d = nc.dram_tensor("out", [NSL, DM], F32, kind="ExternalOutput")

    # [dest, tensor(diff,glob), 128, 128]; a2a1 = chunks 0-7, a2a2 = 8-15
    a2a1_in = nc.dram_tensor("a2a1_in", [NCORES, 2, 128, 128], BF)
    a2a1_out = nc.dram_tensor("a2a1_out", [NCORES, 2, 128, 128], BF)
    a2a2_in = nc.dram_tensor("a2a2_in", [NCORES, 2, 128, 128], BF)
    a2a2_out = nc.dram_tensor("a2a2_out", [NCORES, 2, 128, 128], BF)

    with tile.TileContext(nc) as tc, ExitStack() as root:
        cpool = root.enter_context(tc.tile_pool(name="consts", bufs=1))
        persist = root.enter_context(tc.tile_pool(name="persist", bufs=1))

        ident = cpool.tile([128, 128], BF)
        maskT = cpool.tile([128, 128], BF)
        lmask = cpool.tile([128, 256], BF)
        Ml_sb = cpool.tile([128, L, 128], BF)
        bdWkT = cpool.tile([128, L, 128], BF)
        bdWvT = cpool.tile([128, L, 128], BF)
        ones_row = cpool.tile([1, 128], BF)
        w5b = cpool.tile([128, 5], F32)
        hs = cpool.tile([1, L], F32)

        glob = persist.tile([128, N], F32)
        loc = persist.tile([128, N], BF)
        diff_bf = persist.tile([128, N], BF)
        glob_bf = persist.tile([128, N], BF)
        wg_sb = persist.tile([128, 16, DM], BF)
        wo_sb = persist.tile([128, 8, DM], BF)
        xslT = persist.tile([128, 8, NSL], BF)
        bg_sb = cpool.tile([1, DM], BF)
        bo_sb = cpool.tile([1, DM], BF)
        bgo_sb = cpool.tile([1, 1], BF)
        wgo_sb = cpool.tile([128, 8], BF)

        with ExitStack() as phAB:
            keep = phAB.enter_context(tc.tile_pool(name="keep", bufs=1))
            qT = keep.tile([128, N], BF)
            klT = keep.tile([128, N], BF)
            qpT = keep.tile([128, N], BF)
            kpT = keep.tile([128, N], BF)
            knat = keep.tile([128, N], BF)
            kpnat = keep.tile([128, N], BF)
            vaug = keep.tile([128, 2 * NCH, 65], BF)
            vlaug = keep.tile([128, 2 * NCH, 65], BF)
            vnat = keep.tile([128, N], BF)
            S_sb = keep.tile([128, 5, 65], F32)
            S_bf = keep.tile([128, 5, 65], BF)

            with ExitStack() as phA:
                trans = phA.enter_context(tc.tile_pool(name="trans", bufs=1))
                phX = phA.enter_context(ExitStack())
                xT_p = phX.enter_context(tc.tile_pool(name="xTp", bufs=1))
                wp_p = phX.enter_context(tc.tile_pool(name="wproj", bufs=1))
                ps_mm = phX.enter_context(
                    tc.tile_pool(name="ps_mm", bufs=1, space="PSUM"))

                # projection inputs first; all DMAs are contiguous
                # per-partition copies (host pre-arranged)
                xT = xT_p.tile([128, 8, N], BF)
                wsb = {}
                for k in range(2):
                    nc.sync.dma_start(xT[:, k, :], xT_d[:, k, :])
                for p in ("q", "k", "v", "kl", "vl"):
                    wsb[p] = wp_p.tile([128, 8, 128], BF, tag=f"w{p}",
                                       name=f"wsb_{p}")
                    nc.sync.dma_start(wsb[p][:], wT[p].ap())
                for k in range(2, 8):
                    nc.sync.dma_start(xT[:, k, :], xT_d[:, k, :])

                nc.sync.dma_start(ident[:], ident_d[:])
                nc.sync.dma_start(maskT[:], maskT_d[:])
                nc.sync.dma_start(lmask[:], lmask_d[:])
                nc.sync.dma_start(Ml_sb[:], Ml_d.ap())
                nc.sync.dma_start(bdWkT[:], bdWkT_d.ap())
                nc.sync.dma_start(bdWvT[:], bdWvT_d.ap())
                nc.sync.dma_start(hs[:], hs_d[:])
                nc.vector.memset(ones_row[:], 1.0)

                # tail weights prefetch (have ~250us of slack)
                nc.sync.dma_start(wg_sb[:], wgT_d.ap())
                nc.sync.dma_start(wo_sb[:], woT_d.ap())
                nc.sync.dma_start(xslT[:], xslT_d[:])
                nc.sync.dma_start(bg_sb[:], bg_d[:])
                nc.sync.dma_start(bo_sb[:], bo_d[:])
                nc.sync.dma_start(bgo_sb[:], bgo_d[:])
                nc.sync.dma_start(wgo_sb[:], wgo_d.ap())

                # softmax(haar_scale) -> w5b[:, 1:5], broadcast down columns
                e4 = cpool.tile([1, L], F32)
                s1 = cpool.tile([1, 1], F32)
                nc.scalar.activation(e4[:], hs[:], AF.Exp, accum_out=s1[:])
                r1 = cpool.tile([1, 1], F32)
                nc.vector.reciprocal(r1[:], s1[:])
                w5 = cpool.tile([1, 5], BF)
                nc.vector.memset(w5[:, 0:1], 1.0)
                nc.vector.tensor_scalar_mul(w5[:, 1:5], e4[:], r1[:])
                w5bp = ps_mm.tile([128, 5], F32, tag="w5bp")
                nc.tensor.matmul(w5bp[:], ones_row[:], w5[:],
                                 start=True, stop=True)
                nc.vector.tensor_copy(w5b[:], w5bp[:])

                # ----- projections (k-outer so compute pipelines the DMA) ---
                kTt = trans.tile([128, N], BF)
                vTt = trans.tile([128, N], BF)
                vlTt = trans.tile([128, N], BF)
                for p, dst in (("q", qT), ("k", kTt), ("v", vTt),
                               ("kl", klT), ("vl", vlTt)):
                    accs = [ps_mm.tile([128, 512], F32, tag=f"pacc{nb}",
                                       name=f"acc_{p}_{nb}")
                            for nb in range(4)]
                    for k in range(8):
                        for nb in range(4):
                            nc.tensor.matmul(
                                accs[nb][:], wsb[p][:, k, :],
                                xT[:, k, 512 * nb:512 * (nb + 1)],
                                start=(k == 0), stop=(k == 7))
                    for nb in range(4):
                        nc.any.tensor_copy(dst[:, 512 * nb:512 * (nb + 1)],
                                           accs[nb][:])

                phX.close()
                # allocated only now: must not coexist with xT's 32KB
                kplT_all = keep.tile([128, L, N], BF)
                kplN_all = keep.tile([128, L, N], BF)
                va_all = keep.tile([128, L, 2 * NCH, 65], BF)

                # ----- phi(q), phi(k) + natural layouts via PE transpose ----
                phT = phA.enter_context(ExitStack())
                tmp_p = phT.enter_context(tc.tile_pool(name="phitmp", bufs=2))
                ps_tr = phT.enter_context(
                    tc.tile_pool(name="ps_tr", bufs=3, space="PSUM"))

                def phi_big(dst, src):
                    tmp = tmp_p.tile([128, N], BF, tag="phitmp")
                    nc.vector.tensor_scalar_min(tmp[:], src[:], 0.0)
                    nc.scalar.activation(dst[:], tmp[:], AF.Exp)
                    nc.vector.scalar_tensor_tensor(
                        dst[:], src[:], 0.0, dst[:], op0=OP.max, op1=OP.add)

                phi_big(qpT, qT)
                phi_big(kpT, kTt)

                nc.vector.memset(vaug[:, :, 64:65], 1.0)
                nc.vector.memset(vlaug[:, :, 64:65], 1.0)
                for i in range(NCH):
                    sl = slice(CH * i, CH * (i + 1))
                    pt = ps_tr.tile([128, 128], BF, tag="ptr")
                    nc.tensor.transpose(pt[:], kTt[:, sl], ident[:])
                    nc.any.tensor_copy(knat[:, sl], pt[:])
                    for src, dst in ((vTt, vaug), (vlTt, vlaug)):
                        pt = ps_tr.tile([128, 128], BF, tag="ptr")
                        nc.tensor.transpose(pt[:], src[:, sl], ident[:])
                        for h in range(2):
                            nc.any.tensor_copy(dst[:, 2 * i + h, 0:64],
                                               pt[:, 64 * h:64 * h + 64])
                        if dst is vaug:
                            nc.any.tensor_copy(vnat[:, sl], pt[:])
                # phi commutes with transpose: kpnat = phi(knat)
                phi_big(kpnat, knat)
                phT.close()

                # ----- Haar level tensors, batched across all chunks -------
                phH = phA.enter_context(ExitStack())
                bma_p = phH.enter_context(tc.tile_pool(name="bmall", bufs=2))
                tmpb_p = phH.enter_context(tc.tile_pool(name="tmpb", bufs=2))
                ps_bk = phH.enter_context(
                    tc.tile_pool(name="ps_bk", bufs=2, space="PSUM"))
                ps_bg = phH.enter_context(
                    tc.tile_pool(name="ps_bg", bufs=2, space="PSUM"))

                def phi_blk(dst, psrc):
                    tmp = tmpb_p.tile([128, 512], BF, tag="phb")
                    nc.vector.tensor_scalar_min(tmp[:], psrc, 0.0)
                    nc.scalar.activation(dst, tmp[:], AF.Exp)
                    nc.vector.scalar_tensor_tensor(
                        dst, psrc, 0.0, dst, op0=OP.max, op1=OP.add)

                for lv in range(L):
                    bmk_all = bma_p.tile([128, N], BF, tag="bmk_all")
                    bmv_all = bma_p.tile([128, N], BF, tag="bmv_all")
                    for g in range(4):
                        gs = slice(512 * g, 512 * (g + 1))
                        pk = ps_bk.tile([128, 4, 128], F32, tag="pk")
                        pv = ps_bk.tile([128, 4, 128], F32, tag="pv")
                        for j in range(4):
                            i = 4 * g + j
                            sl = slice(CH * i, CH * (i + 1))
                            nc.tensor.matmul(pk[:, j, :], knat[:, sl],
                                             Ml_sb[:, lv, :],
                                             start=True, stop=True)
                            nc.tensor.matmul(pv[:, j, :], vnat[:, sl],
                                             Ml_sb[:, lv, :],
                                             start=True, stop=True)
                        nc.any.tensor_copy(
                            bmk_all[:, gs],
                            pk[:].rearrange("p a b -> p (a b)"))
                        # fold softmax(haar_scale)[lv] into the v block means
                        nc.scalar.mul(
                            bmv_all[:, gs],
                            pv[:].rearrange("p a b -> p (a b)"),
                            w5b[:, lv + 1:lv + 2])
                    # k_lvl transposed orientation: bdWk^T @ bm, then phi
                    for g in range(4):
                        gs = slice(512 * g, 512 * (g + 1))
                        pT = ps_bg.tile([128, 512], F32, tag="pT")
                        nc.tensor.matmul(pT[:], bdWkT[:, lv, :],
                                         bmk_all[:, gs],
                                         start=True, stop=True)
                        phi_blk(kplT_all[:, lv, gs], pT[:])
                    # k_lvl natural orientation: bm @ bdWk, then phi
                    for g in range(4):
                        gs = slice(512 * g, 512 * (g + 1))
                        pN = ps_bk.tile([128, 4, 128], F32, tag="pk")
                        for j in range(4):
                            i = 4 * g + j
                            sl = slice(CH * i, CH * (i + 1))
                            nc.tensor.matmul(pN[:, j, :], bmk_all[:, sl],
                                             bdWkT[:, lv, :],
                                             start=True, stop=True)
                        phi_blk(kplN_all[:, lv, gs],
                                pN[:].rearrange("p a b -> p (a b)"))
                    # v_lvl natural (already w-scaled via bmv_all)
                    nc.vector.memset(va_all[:, lv, :, 64:65], 1.0)
                    for g in range(4):
                        pV = ps_bk.tile([128, 4, 128], F32, tag="pv")
                        for j in range(4):
                            i = 4 * g + j
                            sl = slice(CH * i, CH * (i + 1))
                            nc.tensor.matmul(pV[:, j, :], bmv_all[:, sl],
                                             bdWvT[:, lv, :],
                                             start=True, stop=True)
                        nc.any.tensor_copy(
                            va_all[:, lv, 8 * g:8 * (g + 1), 0:64],
                            pV[:].rearrange("p a (h d) -> p (a h) d", h=2))
                phH.close()

            # ----- chunk-major recurrence + local attention -----
            atm_p = phAB.enter_context(tc.tile_pool(name="atm", bufs=3))
            tin_p = phAB.enter_context(tc.tile_pool(name="tiny", bufs=4))
            ps_A = phAB.enter_context(
                tc.tile_pool(name="ps_A", bufs=2, space="PSUM"))
            ps_O = phAB.enter_context(
                tc.tile_pool(name="ps_O", bufs=2, space="PSUM"))
            ps_Sd = phAB.enter_context(
                tc.tile_pool(name="ps_Sd", bufs=1, space="PSUM"))
            ps_Lo = phAB.enter_context(
                tc.tile_pool(name="ps_Lo", bufs=2, space="PSUM"))

            def a2a_stage(half, a2a_in_t):
                cs = slice(1024 * half, 1024 * (half + 1))
                nc.vector.tensor_sub(diff_bf[:, cs], loc[:, cs], glob[:, cs])
                nc.vector.tensor_copy(glob_bf[:, cs], glob[:, cs])
                nc.sync.dma_start(
                    a2a_in_t.ap()[:, 0].rearrange("j p m -> p j m"),
                    diff_bf[:, cs].rearrange("p (j m) -> p j m", m=128))
                nc.sync.dma_start(
                    a2a_in_t.ap()[:, 1].rearrange("j p m -> p j m"),
                    glob_bf[:, cs].rearrange("p (j m) -> p j m", m=128))

            for i in range(NCH):
                sl = slice(CH * i, CH * (i + 1))
                psSd = ps_Sd.tile([128, 5, 65], F32, tag="psSd")
                for h in range(2):
                    hp = slice(64 * h, 64 * h + 64)
                    psO = ps_O.tile([128, 5, 65], F32, tag="psO")
                    for lv in range(5):
                        if lv == 0:
                            kpT_l = kpT[hp, sl]
                            va_l = vaug[:, 2 * i + h, :]
                        else:
                            kpT_l = kplT_all[hp, lv - 1, sl]
                            va_l = va_all[:, lv - 1, 2 * i + h, :]
                        psA = ps_A.tile([128, 128], F32, tag="psA")
                        nc.tensor.matmul(psA[:], kpT_l, qpT[hp, sl],
                                         start=True, stop=True)
                        atm = atm_p.tile([128, 128], BF, tag="atm")
                        nc.vector.tensor_mul(atm[:], psA[:], maskT[:])
                        nc.tensor.matmul(psO[:, lv, :], atm[:], va_l,
                                         start=True, stop=(i == 0))
                        if i > 0:
                            nc.tensor.matmul(psO[:, lv, :], qpT[hp, sl],
                                             S_bf[hp, lv, :],
                                             start=False, stop=True)
                    dmax = tin_p.tile([128, 5], F32, tag="dmax")
                    nc.vector.tensor_scalar_max(dmax[:], psO[:, :, 64], EPS)
                    rec = tin_p.tile([128, 5], F32, tag="rec")
                    nc.vector.reciprocal(rec[:], dmax[:])
                    gsl = glob[:, CH * i + 64 * h:CH * i + 64 * h + 64]
                    nc.vector.tensor_scalar_mul(gsl, psO[:, 0, 0:64],
                                                rec[:, 0:1])
                    for lv in range(1, 5):
                        nc.vector.scalar_tensor_tensor(
                            gsl, psO[:, lv, 0:64], rec[:, lv:lv + 1], gsl,
                            op0=OP.mult, op1=OP.add)
                    for lv in range(5):
                        if lv == 0:
                            kn_l = kpnat[:,
                                         CH * i + 64 * h:CH * i + 64 * h + 64]
                            va_l = vaug[:, 2 * i + h, :]
                        else:
                            kn_l = kplN_all[:, lv - 1,
                                            CH * i + 64 * h:CH * i + 64 * h + 64]
                            va_l = va_all[:, lv - 1, 2 * i + h, :]
                        nc.tensor.matmul(psSd[hp, lv, :], kn_l, va_l,
                                         start=True, stop=True)
                if i == 0:
                    nc.vector.tensor_copy(S_sb[:], psSd[:])
                else:
                    nc.vector.tensor_add(S_sb[:], S_sb[:], psSd[:])
                if i < NCH - 1:
                    nc.scalar.copy(S_bf[:], S_sb[:])

                for h in range(2):
                    hp = slice(64 * h, 64 * h + 64)
                    psL = ps_A.tile([128, 256], F32, tag="psA")
                    if i > 0:
                        nc.tensor.matmul(psL[:, 0:128],
                                         klT[hp, CH * (i - 1):CH * i],
                                         qT[hp, sl], start=True, stop=True)
                    nc.tensor.matmul(psL[:, 128:256], klT[hp, sl], qT[hp, sl],
                                     start=True, stop=True)
                    P = atm_p.tile([128, 256], BF, tag="P")
                    if i > 0:
                        nc.scalar.activation(P[:], psL[:], AF.Exp, scale=0.125)
                        nc.vector.tensor_mul(P[:], P[:], lmask[:])
                    else:
                        nc.scalar.activation(P[:, 128:256], psL[:, 128:256],
                                             AF.Exp, scale=0.125)
                        nc.vector.tensor_mul(P[:, 128:256], P[:, 128:256],
                                             lmask[:, 128:256])
                    psLo = ps_Lo.tile([128, 65], F32, tag="psLo")
                    if i > 0:
                        nc.tensor.matmul(psLo[:], P[:, 0:128],
                                         vlaug[:, 2 * (i - 1) + h, :],
                                         start=True, stop=False)
                    nc.tensor.matmul(psLo[:], P[:, 128:256],
                                     vlaug[:, 2 * i + h, :],
                                     start=(i == 0), stop=True)
                    dm = tin_p.tile([128, 1], F32, tag="dm")
                    nc.vector.tensor_scalar_max(dm[:], psLo[:, 64:65], 1e-30)
                    rl = tin_p.tile([128, 1], F32, tag="rl")
                    nc.vector.reciprocal(rl[:], dm[:])
                    nc.scalar.mul(loc[:, CH * i + 64 * h:CH * i + 64 * h + 64],
                                  psLo[:, 0:64], rl[:])

                if i == 7:
                    a2a_stage(0, a2a1_in)
                    nc.gpsimd.collective_compute(
                        "AllToAll", OP.bypass,
                        ins=[a2a1_in.ap().opt()], outs=[a2a1_out.ap().opt()],
                        replica_groups=[list(range(NCORES))])
            a2a_stage(1, a2a2_in)

        nc.gpsimd.collective_compute(
            "AllToAll", OP.bypass,
            ins=[a2a2_in.ap().opt()], outs=[a2a2_out.ap().opt()],
            replica_groups=[list(range(NCORES))])

        # ---------- sequence-parallel tail: two interleaved halves ----------
        with ExitStack() as phC:
            tl = phC.enter_context(tc.tile_pool(name="tail", bufs=2))
            ps_tr2 = phC.enter_context(
                tc.tile_pool(name="ps_tr2", bufs=2, space="PSUM"))
            ps_g = phC.enter_context(
                tc.tile_pool(name="ps_g", bufs=1, space="PSUM"))

            for h2, a2a_out_t in ((0, a2a1_out), (1, a2a2_out)):
                diff_g = tl.tile([128, DM], BF, tag="diff_g")
                glob_g = tl.tile([128, DM], BF, tag="glob_g")
                nc.sync.dma_start(
                    diff_g[:].rearrange("p (s m) -> p s m", s=8),
                    a2a_out_t.ap()[:, 0].rearrange("s p m -> p s m"))
                nc.sync.dma_start(
                    glob_g[:].rearrange("p (s m) -> p s m", s=8),
                    a2a_out_t.ap()[:, 1].rearrange("s p m -> p s m"))

                diffT = tl.tile([128, 8, 128], BF, tag="diffT")
                globT = tl.tile([128, 8, 128], BF, tag="globT")
                for srcT, dstT in ((diff_g, diffT), (glob_g, globT)):
                    for k in range(8):
                        pt2 = ps_tr2.tile([128, 128], BF, tag="ptr2")
                        nc.tensor.transpose(
                            pt2[:], srcT[:, 128 * k:128 * (k + 1)], ident[:])
                        nc.vector.tensor_copy(dstT[:, k, :], pt2[:])

                # gate hidden: [x, diff] @ Wg^T + bg -> silu
                gh = tl.tile([128, DM], BF, tag="gh")
                psG = [ps_g.tile([128, 512], F32, tag=f"psG{g2}",
                                 name=f"psG_{h2}_{g2}") for g2 in range(2)]
                for kc in range(16):
                    lhs = (xslT[:, kc, 128 * h2:128 * (h2 + 1)] if kc < 8
                           else diffT[:, kc - 8, :])
                    for g2 in range(2):
                        nc.tensor.matmul(
                            psG[g2][:], lhs,
                            wg_sb[:, kc, 512 * g2:512 * (g2 + 1)],
                            start=(kc == 0), stop=False)
                for g2 in range(2):
                    nc.tensor.matmul(
                        psG[g2][:], ones_row[:],
                        bg_sb[:, 512 * g2:512 * (g2 + 1)],
                        start=False, stop=True)
                    nc.scalar.activation(
                        gh[:, 512 * g2:512 * (g2 + 1)],
                        psG[g2][:], AF.Silu)

                # oD = diff @ Wo^T (alpha-independent, overlaps alpha chain)
                psFD = [ps_g.tile([128, 512], F32, tag=f"psFD{o2}",
                                  name=f"psFD_{h2}_{o2}") for o2 in range(2)]
                for kc in range(8):
                    for o2 in range(2):
                        nc.tensor.matmul(
                            psFD[o2][:], diffT[:, kc, :],
                            wo_sb[:, kc, 512 * o2:512 * (o2 + 1)],
                            start=(kc == 0), stop=(kc == 7))

                ghT = tl.tile([128, 8, 128], BF, tag="ghT")
                for k in range(8):
                    pt2 = ps_tr2.tile([128, 128], BF, tag="ptr2")
                    nc.tensor.transpose(
                        pt2[:], gh[:, 128 * k:128 * (k + 1)], ident[:])
                    nc.vector.tensor_copy(ghT[:, k, :], pt2[:])

                psAl = ps_tr2.tile([128, 1], F32, tag="psAl", bufs=1)
                for gc in range(8):
                    nc.tensor.matmul(psAl[:], ghT[:, gc, :],
                                     wgo_sb[:, gc:gc + 1],
                                     start=(gc == 0), stop=False)
                nc.tensor.matmul(psAl[:], ones_row[:], bgo_sb[:],
                                 start=False, stop=True)
                alpha = tl.tile([128, 1], F32, tag="alpha")
                nc.scalar.activation(alpha[:], psAl[:], AF.Sigmoid)

                # oG = glob @ Wo^T + bo, reusing the gate's PSUM banks
                psFG = [ps_g.tile([128, 512], F32, tag=f"psG{o2}",
                                  name=f"psFG_{h2}_{o2}") for o2 in range(2)]
                for kc in range(8):
                    for o2 in range(2):
                        nc.tensor.matmul(
                            psFG[o2][:], globT[:, kc, :],
                            wo_sb[:, kc, 512 * o2:512 * (o2 + 1)],
                            start=(kc == 0), stop=False)
                for o2 in range(2):
                    nc.tensor.matmul(
                        psFG[o2][:], ones_row[:],
                        bo_sb[:, 512 * o2:512 * (o2 + 1)],
                        start=False, stop=True)

                # out = alpha * oD + oG (oG staged via SBUF: the DVE
                # cannot read two PSUM operands in one op)
                oG_sb = tl.tile([128, DM], F32, tag="oG_sb")
                out_sb = tl.tile([128, DM], F32, tag="out_sb")
                for o2 in range(2):
                    nc.scalar.copy(oG_sb[:, 512 * o2:512 * (o2 + 1)],
                                   psFG[o2][:])
                for o2 in range(2):
                    nc.vector.scalar_tensor_tensor(
                        out_sb[:, 512 * o2:512 * (o2 + 1)],
                        psFD[o2][:], alpha[:],
                        oG_sb[:, 512 * o2:512 * (o2 + 1)],
                        op0=OP.mult, op1=OP.add)

                nc.sync.dma_start(out_d.ap()[128 * h2:128 * (h2 + 1), :],
                                  out_sb[:])

    nc.compile()
    return nc


def _bf(a):
    return np.asarray(a, dtype=np.float32).astype(ml_dtypes.bfloat16)


def _pkm(a, nk):
    # [nk*128, m] -> [128, nk, m] with row = 128k + p
    m = a.shape[1]
    return np.ascontiguousarray(
        a.reshape(nk, 128, m).transpose(1, 0, 2)).astype(ml_dtypes.bfloat16)


def _prep_in_maps(x, Wq, Wk, Wv, Wkl, Wvl, haar_Wk, haar_Wv, haar_scale,
                  Wg, bg, Wgo, bgo, Wo, bo):
    ident, maskT, lmask, Ml = _host_constants()
    x2 = np.ascontiguousarray(np.asarray(x, dtype=np.float32).reshape(N, DM))
    # xT[p, k, n] = x[n, 128k + p]
    xTf = x2.T.reshape(8, 128, N)
    xT = np.ascontiguousarray(xTf.transpose(1, 0, 2)).astype(
        ml_dtypes.bfloat16)
    bdWkT = np.stack([_blockdiag2(np.asarray(haar_Wk[lv], dtype=np.float32).T)
                      for lv in range(L)])
    bdWvT = np.stack([_blockdiag2(np.asarray(haar_Wv[lv], dtype=np.float32).T)
                      for lv in range(L)])
    wgT = np.ascontiguousarray(np.asarray(Wg, dtype=np.float32).T)
    woT = np.ascontiguousarray(np.asarray(Wo, dtype=np.float32).T)
    wgo = np.asarray(Wgo, dtype=np.float32).reshape(8, 128)
    shared = {
        "xT": xT,
        "bdWkT": _bf(bdWkT.transpose(1, 0, 2)),
        "bdWvT": _bf(bdWvT.transpose(1, 0, 2)),
        "Ml": _bf(Ml.transpose(1, 0, 2)),
        "maskT": _bf(maskT), "lmask": _bf(lmask), "ident": _bf(ident),
        "wgT": _pkm(wgT, 16), "woT": _pkm(woT, 8),
        "wgo": _bf(wgo.T),
        "bg": _bf(np.asarray(bg, dtype=np.float32).reshape(1, DM)),
        "bo": _bf(np.asarray(bo, dtype=np.float32).reshape(1, DM)),
        "bgo": _bf(np.asarray(bgo, dtype=np.float32).reshape(1, 1)),
        "hscale": np.asarray(haar_scale, dtype=np.float32).reshape(1, L),
    }
    in_maps = []
    for c in range(NCORES):
        sc = slice(128 * c, 128 * (c + 1))
        m = dict(shared)
        for nm, W in (("wqT", Wq), ("wkT", Wk), ("wvT", Wv),
                      ("wklT", Wkl), ("wvlT", Wvl)):
            m[nm] = _pkm(np.ascontiguousarray(
                np.asarray(W, dtype=np.float32)[sc, :].T), 8)
        # core c owns chunks c and 8+c: xslT[p, k, 128*half + r]
        m["xslT"] = np.ascontiguousarray(np.concatenate(
            [xTf[:, :, 128 * c:128 * (c + 1)],
             xTf[:, :, 128 * (8 + c):128 * (9 + c)]],
            axis=2).transpose(1, 0, 2)).astype(ml_dtypes.bfloat16)
        in_maps.append(m)
    return in_maps


def kernel_run(inputs, trace=False):
    if "nc" not in _CACHE:
        _CACHE["nc"] = _build_nc()
    nc = _CACHE["nc"]
    in_maps = _prep_in_maps(**inputs)
    res = run_bass_kernel_spmd(nc, in_maps, list(range(NCORES)), trace=trace)
    out = np.zeros((N, DM), dtype=np.float32)
    for c in range(NCORES):
        r = res.results[c]["out"]
        out[128 * c:128 * (c + 1)] = r[0:128]
        out[128 * (8 + c):128 * (9 + c)] = r[128:256]
    return out.reshape(1, N, DM), res


def kernel(**inputs):
    out, _ = kernel_run(inputs, trace=False)
    return out
